# revision 31
# baseline (speedup 1.0000x reference)
"""Bahdanau additive attention scores on 8 TRN2 NeuronCores.

reference:
    h = hidden[-1]                                   # [B, He]
    e_proj = enc @ W_e;  h_proj = h @ W_h            # W_attn = [W_h; W_e]
    scores = tanh(h_proj[:,None,:] + e_proj + b) @ v # [B, S]
    out = softmax(scores, axis=1)

Graded mode "dr1024" (MODE below): the whole GEMM runs as fp8e4m3 with
perf_mode=DoubleRow -- K=256 channels per instruction, 2 MACs/cell/cycle
-- for 1024 DR matmuls/core vs 2048 f32r ones. Measured 307-315 us vs
544-582 us for the f32r baseline (~1.8x), rel err 1.780e-2 vs the 2e-2
gate. The error is bit-deterministic (verified identical across
independent compiles+runs: fixed inputs, fixed instruction order, fixed
HW numerics), so the 11% margin is real, not statistical.

fp8 specifics:
  - W_e pre-scaled x1024 (pow2) before the e4m3 cast so sigma=0.022
    weights clear the e4m3 subnormal zone (min normal 2^-6); descale is
    folded into the tanh ACT as scale=1/1024. Direct-cast err 2.1e-2 ->
    scaled 1.9e-2 (offline study, exact emulation).
  - enc cast e4m3 unscaled (sigma=1 is already well-placed; scaling
    measured no-op).
  - Error floor is the e4m3 3-bit mantissa; DR's internal e6m3 pair
    format means no fp8 variant beats it. Hybrid f32r/fp8 modes
    ("dr768": 1.555e-2 @ 374us, "dr512": ~1.3e-2) trade time for margin.
  - s-blocks processed in groups of 3 so each DR weight tile is reused
    across 3 consecutive matmuls (amortizes DR's 256-col LDWEIGHTS);
    measured worth ~1% (_ki A/B) -- LDW is almost fully hidden anyway.
  - PE-bound end to end: GEMM-only probe (_pe) is 297.4us vs 306.9 full,
    and the delta is almost exactly the 32 score-reduction matmuls.
    Non-PE exposure ~1.3us total.

Previous f32r strategy (modes "full_dvesc" etc., kept below and still
selectable): pure data-parallel over batch (B=32 -> 4 per core), zero
collectives. Host-side prep (free, off the HW critical path):
  - c = h @ W_h + b_attn  folded into a per-(batch, hd-tile) bias vector
  - encoder shard pre-transposed to [b, He, S] so the contraction dim He
    lands on SBUF partitions with no on-device transposes
  - W_e pre-tiled to [128, (k, hd, m)] so each [K=128, M=128] lhsT tile is
    a contiguous slice

Device program per core (TileContext), per (batch, 512-col s-block):
  e_projT[hd] = sum_k W_e[k,hd].T @ encT[k]   8x8 f32r matmuls -> PSUM
                (f32r = tf32-like: 1 col/cycle vs 4 for fp32; measured
                 ~0.57 ns/col on this fleet, rel err ~1e-4)
  th[hd]  = tanh(e_projT[hd] + c[b,hd])       ScalarE, PSUM->SBUF, fused bias
  acc    += th[hd] * v[hd]                    VectorE scalar_tensor_tensor
  scores  = ones.T @ acc                      1 matmul: partition-reduce
  exp_row[s-block], partial = Exp(scores)     ScalarE with accum_out
The scores reduction + Exp for block i are deferred into block i+1's
matmul stream so PE never waits on ScalarE/VectorE results.
Per batch: total = sum(partials); the final scaling runs on ScalarE
(fused scale=1/total) in two chunks so the first chunk's output DMA
overlaps the second chunk's scaling.

Softmax skips the max-subtraction: scores are ~N(0, 0.65), |max| < ~4
over 128K samples, exp() is comfortably within f32 range.

Measured: ~490 us per invocation on a quiet chip, ~590 us under fleet
load (per-matmul cost is sequencer-bound ~230 ns quiet / stream-bound
~300 ns busy; 2080 matmuls is the information-theoretic minimum at the
K<=128, N<=512 instruction caps). ~70 TF/s/core effective on the
275 GFLOP GEMM. rel err 9.6e-5.

Rejected-but-measured alternatives (kept as modes for the record):
  - fp8e4 DoubleRow whole-GEMM ("fp8dr"): 1024 instructions, ~1.6x, but
    rel err 1.95e-2 sits on the 2e-2 gate.
  - hybrid He[0:256] fp8-DR + He[256:1024] f32r ("hyb"): 1792
    instructions, ~5-10% faster, rel err 1.00e-2 -- spends half the
    error budget for <10% speed; not worth it.
  - bf16, interleaved accumulation groups, single-group, PSUM buffer
    counts, weight reuse: all within noise of the f32r baseline.

build_nc(n_loop=N) wraps the body in an in-NEFF For_i loop -- used by
test.py to amortize the ~80 ms axon-tunnel dispatch cost when timing.
The graded path is build_nc() defaults.
"""

import os

import numpy as np

import concourse.mybir as mybir
import concourse.tile as tile
from concourse import bacc
from concourse.bass_utils import run_bass_kernel_spmd

N_CORES = 8
L, B, S, He, Hd = 2, 32, 4096, 1024, 1024
BPC = B // N_CORES  # batches per core
KT = He // 128      # contraction tiles
HT = Hd // 128      # hd tiles
SB = 512            # s-block (matmul moving free dim)
NSB = S // SB
F32 = mybir.dt.float32
F32R = mybir.dt.float32r
BF16 = mybir.dt.bfloat16
F16 = mybir.dt.float16

# matmul-input dtype for the big GEMM:
#   "f32r"  - tf32-like, 1 col/cycle
#   "bf16"  - 1 col/cycle
#   "fp8dr" - float8e4 with DoubleRow: K=256 per instruction, 2 multiplies/cycle
MM_DTYPE = "f32r"
# v-dot on DVE ("dvesc" mode) keeps v in f32
VW_F32 = True
F8 = mybir.dt.float8e4
KT2 = KT // 2  # 256-deep contraction tiles for DoubleRow

# --- dr-hybrid mode ("dr<k8>[_ki][_pe]"): first k8 channels fp8e4+DoubleRow,
# rest f32r, s-blocks processed in groups so DR weight tiles are reused
# across consecutive matmuls (amortizes the 2x LDWEIGHTS cost of DR).
#   _ki: k-inner loop order (weights change every matmul) -- A/B probe
#   _pe: emit only the GEMM (no ACT/DVE/softmax) -- PE-bound probe
MODE = os.environ.get("K_MODE", "dr1024")
W_SCALE = 1024.0  # pow2; keeps fp8(W) out of the subnormal zone
DR_GROUPS = ((0, 1, 2), (3, 4, 5), (6, 7))

_NC_CACHE = {}


def _dr_k8(mode):
    return int(mode[2:].split("_")[0])


def _mm_dt():
    return BF16 if MM_DTYPE == "bf16" else F32R


def _emit_body(nc, pools, params, batches=None, mode="full"):
    AFT = mybir.ActivationFunctionType
    enc_pool, th_pool, soft_pool, ep_pool, sc_pool = pools
    encT, out, w_sb, v_sb, c_sb, ones_sb, et_shared, wh, lazy_w = params[:9]
    hyb = "hyb" in mode
    if hyb:
        encT8, w8_sb = params[9:]
    batches = list(range(BPC)) if batches is None else batches
    fp8 = MM_DTYPE == "fp8dr"
    th_dt = F32 if "dvesc" in mode else _mm_dt()

    # flat list of (batch, s-block); scores finalization for block i is
    # deferred into block i+1 so PE never waits on ACT/DVE results
    blocks = [(b, isb) for b in batches for isb in range(NSB)]
    soft = {}    # b -> (exp_row, parts)
    deferred = None  # (b, isb, sc_or_acc, ths)

    def finish_block(dfr):
        b, isb, acc, ths = dfr
        exp_row, parts = soft[b]
        if "dvesc" in mode:
            sc = sc_pool.tile([1, SB], F32, tag="sc")
            nc.tensor.matmul(sc, ones_sb, acc, start=True, stop=True)
        else:
            sc = sc_pool.tile([1, SB], F32, tag="sc")
            for hd in range(HT):
                nc.tensor.matmul(sc, v_sb[:, hd:hd + 1], ths[hd],
                                 start=(hd == 0), stop=(hd == HT - 1))
        nc.scalar.activation(
            exp_row[:, isb * SB:(isb + 1) * SB], sc, AFT.Exp,
            accum_out=parts[:, isb:isb + 1])
        if isb == NSB - 1:
            # batch done: softmax normalization + output
            tot = soft_pool.tile([1, 1], F32, tag="tot")
            nc.vector.tensor_reduce(tot, parts, axis=mybir.AxisListType.X,
                                    op=mybir.AluOpType.add)
            rinv = soft_pool.tile([1, 1], F32, tag="rinv")
            nc.vector.reciprocal(rinv, tot)
            # scale on ScalarE (1.2 GHz vs DVE 0.96 single-lane), in two
            # chunks so the first chunk's output DMA overlaps the second
            # chunk's scaling -- trims the exposed final-batch tail
            half = S // 2
            for c2 in range(2):
                oc = soft_pool.tile([1, half], F32, tag="oc", bufs=4,
                                    name=f"oc_{b}_{c2}")
                nc.scalar.activation(oc, exp_row[:, c2 * half:(c2 + 1) * half],
                                     AFT.Copy, scale=rinv)
                nc.sync.dma_start(out=out[b:b + 1, c2 * half:(c2 + 1) * half],
                                  in_=oc)
            del soft[b]

    for b, isb in blocks:
        if b not in soft:
            soft[b] = (soft_pool.tile([1, S], F32, tag="exp_row",
                                      name=f"exp_row_{b}"),
                       soft_pool.tile([1, NSB], F32, tag="parts",
                                      name=f"parts_{b}"))
        if "compute" in mode:
            et = et_shared
        elif hyb:
            et8 = enc_pool.tile([128, 2, SB], F8, tag="et8")
            nc.sync.dma_start(
                out=et8, in_=encT8[b, :, :, isb * SB:(isb + 1) * SB])
            et = []
            for k in range(KT - 2):
                t = enc_pool.tile([128, SB], F32R, tag="et")
                nc.sync.dma_start(
                    out=t,
                    in_=encT[b, k * 128:(k + 1) * 128, isb * SB:(isb + 1) * SB])
                et.append(t)
        elif fp8:
            et = []
            for k2 in range(KT2):
                t = enc_pool.tile([128, 2, SB], F8, tag="et")
                nc.sync.dma_start(
                    out=t, in_=encT[b, k2, :, :, isb * SB:(isb + 1) * SB])
                et.append(t)
        elif "wet" in mode:
            # wide et: one [128, 2*SB] tile per k covers two s-blocks --
            # halves DMA count and first-use sem waits
            first = (b, isb) == blocks[0]
            if isb % 2 == 0:
                etw = []
                for k in range(KT):
                    t = enc_pool.tile([128, 2 * SB], _mm_dt(), tag="etw",
                                      bufs=12, name=f"etw{k}")
                    nc.sync.dma_start(
                        out=t,
                        in_=encT[b, k * 128:(k + 1) * 128,
                                 isb * SB:(isb + 2) * SB])
                    etw.append(t)
                    if lazy_w and first:
                        ck = HT * 128
                        nc.sync.dma_start(out=w_sb[:, k * ck:(k + 1) * ck],
                                          in_=wh[:, k * ck:(k + 1) * ck])
                _emit_body.etw = etw
            off = (isb % 2) * SB
            et = [t[:, off:off + SB] for t in _emit_body.etw]
        else:
            first = (b, isb) == blocks[0]
            et = []
            for k in range(KT):
                t = enc_pool.tile([128, SB], _mm_dt(), tag="et")
                nc.sync.dma_start(
                    out=t,
                    in_=encT[b, k * 128:(k + 1) * 128, isb * SB:(isb + 1) * SB])
                et.append(t)
                if lazy_w and first:
                    ck = HT * 128
                    nc.sync.dma_start(out=w_sb[:, k * ck:(k + 1) * ck],
                                      in_=wh[:, k * ck:(k + 1) * ck])

        acc = None
        ths = []
        if "g1" in mode:
            first_blk = (b, isb) == blocks[0]
            last_blk = (b, isb) == blocks[-1]
            ep = ep_pool.tile([128, SB], F32, tag="ep1", bufs=1,
                              name="ep_g1")
            for hd in range(HT):
                for k in range(KT):
                    w_tile = w_sb[:, (k * HT + hd) * 128:(k * HT + hd + 1) * 128]
                    nc.tensor.matmul(
                        ep, w_tile, et[k],
                        start=(first_blk and hd == 0 and k == 0),
                        stop=(last_blk and hd == HT - 1 and k == KT - 1))
            if last_blk:
                probe = soft_pool.tile([128, 1], F32, tag="probe")
                nc.scalar.activation(probe, ep[:, 0:1], AFT.Copy)
            continue
        for hd in range(HT):
            ep = ep_pool.tile([128, SB], F32, tag="ep")
            if fp8:
                for k2 in range(KT2):
                    nc.tensor.matmul(
                        ep, w_sb[:, k2 * HT + hd, :, :], et[k2],
                        start=(k2 == 0), stop=(k2 == KT2 - 1),
                        perf_mode=mybir.MatmulPerfMode.DoubleRow)
            elif hyb:
                nc.tensor.matmul(ep, w8_sb[:, hd, :, :], et8,
                                 start=True, stop=False,
                                 perf_mode=mybir.MatmulPerfMode.DoubleRow)
                for k in range(KT - 2):
                    w_tile = w_sb[:, (k * HT + hd) * 128:(k * HT + hd + 1) * 128]
                    nc.tensor.matmul(ep, w_tile, et[k],
                                     start=False, stop=(k == KT - 3))
            else:
                kr = range(KT // 2) if "k4" in mode else range(KT)
                for k in kr:
                    w_tile = w_sb[:, (k * HT + hd) * 128:(k * HT + hd + 1) * 128]
                    nc.tensor.matmul(ep, w_tile, et[k],
                                     start=(k == 0),
                                     stop=(k == list(kr)[-1]))
            if hd == 1 and deferred is not None:
                # PE work for the previous block's scores goes here, long
                # after its inputs are ready
                finish_block(deferred)
                deferred = None
            if "noact" in mode:
                if hd == HT - 1:
                    probe = soft_pool.tile([128, 1], F32, tag="probe")
                    nc.scalar.activation(probe, ep[:, 0:1], AFT.Copy)
                continue
            th = th_pool.tile([128, SB], th_dt, tag="th")
            nc.scalar.activation(
                th, ep, AFT.Tanh, bias=c_sb[:, b * HT + hd: b * HT + hd + 1])
            ths.append(th)
            if "dvesc" in mode:
                if hd == 0:
                    acc = th_pool.tile([128, SB], F32, tag="acc", bufs=3)
                    nc.vector.tensor_scalar_mul(acc, th, v_sb[:, 0:1])
                else:
                    nc.vector.scalar_tensor_tensor(
                        acc, th, v_sb[:, hd:hd + 1], acc,
                        op0=mybir.AluOpType.mult, op1=mybir.AluOpType.add)
        if "noact" in mode:
            continue
        if "dvesc" in mode:
            acc8 = th_pool.tile([128, SB], F32R, tag="acc8", bufs=3)
            nc.scalar.activation(acc8, acc, AFT.Copy)
            acc = acc8
        deferred = (b, isb, acc if "dvesc" in mode else None, ths)
    if deferred is not None and "noact" not in mode:
        finish_block(deferred)


def _emit_dr_body(nc, pools, params, batches=None, mode="dr768"):
    AFT = mybir.ActivationFunctionType
    DR = mybir.MatmulPerfMode.DoubleRow
    enc_pool, th_pool, soft_pool, ep_pool, sc_pool = pools
    (enc8, encf, out, w8_sb, wf_sb, v_sb, ones_sb, c_sb, w8p, wfp,
     lazy_w, ones2_sb) = params
    batches = list(range(BPC)) if batches is None else batches
    k8 = _dr_k8(mode)
    kt2 = k8 // 256
    kf = (He - k8) // 128
    ki = "_ki" in mode
    probe_pe = "_pe" in mode
    tt = "_tt" in mode  # transposed softmax tail: no per-block sc matmuls
    ds = 1.0 / W_SCALE

    blocks = [(b, g) for b in batches for g in range(len(DR_GROUPS))]
    soft = {}
    finq = []  # deferred (b, isb, accT) score finishers / tt batch ends

    def finish_batch_tt():
        b = finq.pop(0)
        exp_t, parts_t = soft[b]
        # ones2 is [128,128]: every output partition gets the per-isb
        # cross-partition totals in one N=8 matmul
        tot8 = sc_pool.tile([128, NSB], F32, tag="tot8")
        nc.tensor.matmul(tot8, ones2_sb, parts_t, start=True, stop=True)
        tot = soft_pool.tile([128, 1], F32, tag="tot128")
        nc.vector.tensor_reduce(tot, tot8, axis=mybir.AxisListType.X,
                                op=mybir.AluOpType.add)
        rinv = soft_pool.tile([128, 1], F32, tag="rinv128")
        nc.vector.reciprocal(rinv, tot)
        oc = soft_pool.tile([128, S // 128], F32, tag="oct", bufs=4,
                            name=f"oct_{b}")
        nc.scalar.activation(oc, exp_t, mybir.ActivationFunctionType.Copy,
                             scale=rinv)
        nc.sync.dma_start(
            out=out[b:b + 1, :].rearrange("a (c p) -> (a p) c", p=128),
            in_=oc)
        del soft[b]

    def finish_one():
        b, isb, acc = finq.pop(0)
        exp_row, parts = soft[b]
        sc = sc_pool.tile([1, SB], F32, tag="sc")
        nc.tensor.matmul(sc, ones_sb, acc, start=True, stop=True)
        nc.scalar.activation(
            exp_row[:, isb * SB:(isb + 1) * SB], sc, AFT.Exp,
            accum_out=parts[:, isb:isb + 1])
        if isb == NSB - 1:
            tot = soft_pool.tile([1, 1], F32, tag="tot")
            nc.vector.tensor_reduce(tot, parts, axis=mybir.AxisListType.X,
                                    op=mybir.AluOpType.add)
            rinv = soft_pool.tile([1, 1], F32, tag="rinv")
            nc.vector.reciprocal(rinv, tot)
            half = S // 2
            for c2 in range(2):
                oc = soft_pool.tile([1, half], F32, tag="oc", bufs=4,
                                    name=f"oc_{b}_{c2}")
                nc.scalar.activation(oc, exp_row[:, c2 * half:(c2 + 1) * half],
                                     AFT.Copy, scale=rinv)
                nc.sync.dma_start(out=out[b:b + 1, c2 * half:(c2 + 1) * half],
                                  in_=oc)
            del soft[b]

    for b, g in blocks:
        isbs = DR_GROUPS[g]
        ni = len(isbs)
        s0 = isbs[0] * SB
        gw = ni * SB
        if b not in soft and not probe_pe:
            if tt:
                soft[b] = (soft_pool.tile([128, S // 128], F32, tag="exp_t",
                                          name=f"exp_t_{b}"),
                           soft_pool.tile([128, NSB], F32R, tag="parts_t",
                                          name=f"parts_t_{b}"))
            else:
                soft[b] = (soft_pool.tile([1, S], F32, tag="exp_row",
                                          name=f"exp_row_{b}"),
                           soft_pool.tile([1, NSB], F32, tag="parts",
                                          name=f"parts_{b}"))
        first = (b, g) == blocks[0]
        et8 = []
        for k2 in range(kt2):
            t = enc_pool.tile([128, 2, gw], F8, tag=f"et8_{k2}", bufs=3)
            nc.sync.dma_start(out=t, in_=enc8[b, k2, :, :, s0:s0 + gw])
            et8.append(t)
            if lazy_w and first:
                nc.sync.dma_start(
                    out=w8_sb[:, k2 * HT:(k2 + 1) * HT, :, :],
                    in_=w8p[:, k2 * HT:(k2 + 1) * HT, :, :])
        etf = []
        for k in range(kf):
            t = enc_pool.tile([128, gw], F32R, tag=f"etf_{k}", bufs=3)
            nc.sync.dma_start(out=t, in_=encf[b, k * 128:(k + 1) * 128,
                                             s0:s0 + gw])
            etf.append(t)
            if lazy_w and first:
                ck = HT * 128
                nc.sync.dma_start(out=wf_sb[:, k * ck:(k + 1) * ck],
                                  in_=wfp[:, k * ck:(k + 1) * ck])
        if "_sg" in mode:
            # pure-stream probe: one PSUM accumulation group, constant
            # weight+rhs, no ACT/DVE -- measures the intrinsic DR matmul
            # stream floor (PSUM cycling + dep-wait cost excluded)
            first_blk = (b, g) == blocks[0]
            last_blk = (b, g) == blocks[-1]
            epg = ep_pool.tile([128, SB], F32, tag="epg", bufs=1,
                               name="ep_sg")
            nmm_all = HT * ni * kt2
            mi = 0
            for hd in range(HT):
                for k2 in range(kt2):
                    for i in range(ni):
                        nc.tensor.matmul(
                            epg, w8_sb[:, 0, :, :], et8[0][:, :, 0:SB],
                            start=(first_blk and mi == 0),
                            stop=(last_blk and mi == nmm_all - 1),
                            perf_mode=DR)
                        mi += 1
            if last_blk:
                pr = soft_pool.tile([128, 1], F32, tag="probe")
                nc.scalar.activation(pr, epg[:, 0:1], AFT.Copy)
            continue
        accs = [None] * ni
        for hd in range(HT):
            eps = []
            for i in range(ni):
                ep = ep_pool.tile([128, SB], F32, tag="ep", name=f"ep{i}")
                eps.append(ep)
            nmm = kt2 + kf

            def mm(i, mi, k2=None, k=None):
                if k2 is not None:
                    if "_1w" in mode:  # probe: constant weight+rhs tiles
                        nc.tensor.matmul(
                            eps[i], w8_sb[:, 0, :, :], et8[0][:, :, 0:SB],
                            start=(mi == 0), stop=(mi == nmm - 1),
                            perf_mode=DR)
                        return
                    nc.tensor.matmul(
                        eps[i], w8_sb[:, k2 * HT + hd, :, :],
                        et8[k2][:, :, i * SB:(i + 1) * SB],
                        start=(mi == 0), stop=(mi == nmm - 1), perf_mode=DR)
                else:
                    nc.tensor.matmul(
                        eps[i],
                        wf_sb[:, (k * HT + hd) * 128:(k * HT + hd + 1) * 128],
                        etf[k][:, i * SB:(i + 1) * SB],
                        start=(mi == 0), stop=(mi == nmm - 1))

            if ki:  # weights swapped every matmul (A/B probe)
                for i in range(ni):
                    for mi, k2 in enumerate(range(kt2)):
                        mm(i, mi, k2=k2)
                    for mi, k in enumerate(range(kf)):
                        mm(i, kt2 + mi, k=k)
            else:   # weight tile reused across the group's s-blocks
                for mi, k2 in enumerate(range(kt2)):
                    for i in range(ni):
                        mm(i, mi, k2=k2)
                for mi, k in enumerate(range(kf)):
                    for i in range(ni):
                        mm(i, kt2 + mi, k=k)

            if probe_pe:
                if (b, g) == blocks[-1] and hd == HT - 1:
                    pr = soft_pool.tile([128, 1], F32, tag="probe")
                    nc.scalar.activation(pr, eps[-1][:, 0:1], AFT.Copy)
                continue
            if hd >= 1 and finq:
                finish_batch_tt() if tt else finish_one()
            for i in range(ni):
                th = th_pool.tile([128, SB], F32, tag="th")
                nc.scalar.activation(
                    th, eps[i], AFT.Tanh,
                    bias=c_sb[:, b * HT + hd:b * HT + hd + 1], scale=ds)
                if hd == 0:
                    acc = th_pool.tile([128, SB], F32 if tt else F32R,
                                       tag="acc", bufs=8, name=f"acc{i}")
                    accs[i] = acc
                    nc.vector.tensor_scalar_mul(accs[i], th, v_sb[:, 0:1])
                else:
                    nc.vector.scalar_tensor_tensor(
                        accs[i], th, v_sb[:, hd:hd + 1], accs[i],
                        op0=mybir.AluOpType.mult, op1=mybir.AluOpType.add)
        if probe_pe:
            continue
        if tt:
            exp_t, parts_t = soft[b]
            for i, isb in enumerate(isbs):
                a16 = th_pool.tile([128, SB], F16, tag="a16", bufs=4,
                                   name=f"a16_{i}")
                nc.scalar.activation(a16, accs[i],
                                     mybir.ActivationFunctionType.Copy)
                ttile = th_pool.tile([128, SB // 128, 128], F16, tag="ttile",
                                     bufs=4, name=f"ttile_{i}")
                for c in range(SB // 128):
                    nc.sync.dma_start_transpose(
                        ttile[:, c, :], a16[:, c * 128:(c + 1) * 128])
                st = th_pool.tile([128, SB // 128], F32, tag="st", bufs=4,
                                  name=f"st_{i}")
                with nc.allow_low_precision(
                        reason="fp16 transpose staging adds ~0.05% to "
                               "scores; fp32 reduce output"):
                    nc.vector.tensor_reduce(st, ttile,
                                            axis=mybir.AxisListType.X,
                                            op=mybir.AluOpType.add)
                with nc.allow_low_precision(
                        reason="per-block exp partials rounded to f32r for "
                               "the broadcast matmul; 2^-12 relative"):
                    nc.scalar.activation(
                        exp_t[:, isb * 4:(isb + 1) * 4], st,
                        mybir.ActivationFunctionType.Exp,
                        accum_out=parts_t[:, isb:isb + 1])
            if isbs[-1] == NSB - 1:
                finq.append(b)
        else:
            for i, isb in enumerate(isbs):
                finq.append((b, isb, accs[i]))
    while finq:
        finish_batch_tt() if tt else finish_one()


def _build_nc_dr(n_loop, batches, mode, ep_bufs):
    k8 = _dr_k8(mode)
    kt2 = k8 // 256
    kf = (He - k8) // 128
    nc = bacc.Bacc(trn_type="TRN2", target_bir_lowering=False, debug=False,
                   num_devices=N_CORES)
    enc8 = nc.declare_dram_parameter("enc8", [BPC, kt2, 128, 2, S], F8,
                                     isOutput=False)
    w8p = nc.declare_dram_parameter("w8", [128, kt2 * HT, 2, 128], F8,
                                    isOutput=False)
    encf = wfp = None
    if kf:
        encf = nc.declare_dram_parameter("encf", [BPC, kf * 128, S], F32R,
                                         isOutput=False)
        wfp = nc.declare_dram_parameter("wf", [128, kf * HT * 128], F32R,
                                        isOutput=False)
    cb = nc.declare_dram_parameter("cb", [128, BPC * HT], F32, isOutput=False)
    vw = nc.declare_dram_parameter("vw", [128, HT], F32, isOutput=False)
    onesp = nc.declare_dram_parameter("ones", [128, 1], F32R, isOutput=False)
    ones2p = nc.declare_dram_parameter("ones2", [128, 128], F32R,
                                       isOutput=False)
    out = nc.declare_dram_parameter("out", [BPC, S], F32, isOutput=True)

    with tile.TileContext(nc) as tc:
        with (
            tc.tile_pool(name="consts", bufs=1) as consts,
            tc.tile_pool(name="enc", bufs=2) as enc_pool,
            tc.tile_pool(name="th", bufs=10) as th_pool,
            tc.tile_pool(name="soft", bufs=2) as soft_pool,
            tc.tile_pool(name="ep", bufs=ep_bufs, space="PSUM") as ep_pool,
            tc.tile_pool(name="sc", bufs=2, space="PSUM") as sc_pool,
        ):
            lazy_w = n_loop == 1
            w8_sb = consts.tile([128, kt2 * HT, 2, 128], F8)
            if not lazy_w:
                nc.sync.dma_start(out=w8_sb, in_=w8p[:])
            wf_sb = None
            if kf:
                wf_sb = consts.tile([128, kf * HT * 128], F32R)
                if not lazy_w:
                    ck = HT * 128
                    for k in range(kf):
                        nc.sync.dma_start(out=wf_sb[:, k * ck:(k + 1) * ck],
                                          in_=wfp[:, k * ck:(k + 1) * ck])
            v_sb = consts.tile([128, HT], F32)
            nc.sync.dma_start(out=v_sb, in_=vw[:])
            ones_sb = consts.tile([128, 1], F32R)
            nc.sync.dma_start(out=ones_sb, in_=onesp[:])
            ones2_sb = consts.tile([128, 128], F32R)
            nc.sync.dma_start(out=ones2_sb, in_=ones2p[:])
            c_sb = consts.tile([128, BPC * HT], F32)
            nc.sync.dma_start(out=c_sb, in_=cb[:])

            pools = (enc_pool, th_pool, soft_pool, ep_pool, sc_pool)
            params = (enc8, encf, out, w8_sb, wf_sb, v_sb, ones_sb, c_sb,
                      w8p, wfp, lazy_w, ones2_sb)
            if n_loop == 1:
                _emit_dr_body(nc, pools, params, batches, mode)
            else:
                with tc.For_i(0, n_loop, 1):
                    _emit_dr_body(nc, pools, params, batches, mode)
    nc.compile()
    return nc


def prepare_in_maps_dr(hidden, encoder_outputs, W_attn, b_attn, v_w,
                       mode=None):
    import ml_dtypes
    f8 = ml_dtypes.float8_e4m3
    mode = MODE if mode is None else mode
    k8 = _dr_k8(mode)
    kt2 = k8 // 256
    kf = (He - k8) // 128
    hidden = np.ascontiguousarray(np.asarray(hidden, dtype=np.float32))
    enc = np.asarray(encoder_outputs, dtype=np.float32)
    W_attn = np.asarray(W_attn, dtype=np.float32)
    b_attn = np.asarray(b_attn, dtype=np.float32)
    v_w = np.asarray(v_w, dtype=np.float32)

    h = hidden[-1]
    W_h = W_attn[:He]
    W_e = W_attn[He:]
    c = (h @ W_h + b_attn).astype(np.float32)   # [B, Hd]

    # w8[p, k2*HT+hd, j, m] = SW * W_e[k2*256 + j*128 + p, hd*128 + m]
    w8 = np.ascontiguousarray(
        (W_e[:k8] * W_SCALE).reshape(kt2, 2, 128, HT, 128)
        .transpose(2, 0, 3, 1, 4).reshape(128, kt2 * HT, 2, 128).astype(f8))
    wf = None
    if kf:
        # wf[p, (k*HT+hd)*128+m] = SW * W_e[k8 + k*128 + p, hd*128 + m]
        wf = np.ascontiguousarray(
            (W_e[k8:] * W_SCALE).reshape(kf, 128, HT, 128)
            .transpose(1, 0, 2, 3).reshape(128, -1).astype(np.float32))
    vw = np.ascontiguousarray(v_w.reshape(HT, 128).T.astype(np.float32))

    in_maps = []
    for ci in range(N_CORES):
        bsl = slice(ci * BPC, (ci + 1) * BPC)
        # enc8[b, k2, p, j, s] = enc[b, s, k2*256 + j*128 + p]
        enc8 = np.ascontiguousarray(
            enc[bsl, :, :k8].reshape(BPC, S, kt2, 2, 128)
            .transpose(0, 2, 4, 3, 1).astype(f8))
        cbm = np.ascontiguousarray(
            c[bsl].reshape(BPC, HT, 128).transpose(2, 0, 1).reshape(128, -1))
        m = {"enc8": enc8, "w8": w8, "cb": cbm, "vw": vw,
             "ones": np.ones((128, 1), np.float32),
             "ones2": np.ones((128, 128), np.float32)}
        if kf:
            m["encf"] = np.ascontiguousarray(
                enc[bsl, :, k8:].transpose(0, 2, 1).astype(np.float32))
            m["wf"] = wf
        in_maps.append(m)
    return in_maps


def build_nc(n_loop=1, batches=None, mode=None, ep_bufs=None):
    mode = MODE if mode is None else mode
    if ep_bufs is None:
        ep_bufs = 6 if mode.startswith("dr") else 4
    key = (MM_DTYPE, n_loop, tuple(batches) if batches else None, mode, ep_bufs)
    if key in _NC_CACHE:
        return _NC_CACHE[key]
    if mode.startswith("dr"):
        nc = _build_nc_dr(n_loop, batches, mode, ep_bufs)
        _NC_CACHE[key] = nc
        return nc
    return _build_nc_orig(n_loop, batches, mode, ep_bufs)


def _build_nc_orig(n_loop=1, batches=None, mode="full_dvesc", ep_bufs=4):
    key = (MM_DTYPE, n_loop, tuple(batches) if batches else None, mode, ep_bufs)
    if key in _NC_CACHE:
        return _NC_CACHE[key]
    nc = bacc.Bacc(trn_type="TRN2", target_bir_lowering=False, debug=False,
                   num_devices=N_CORES)
    if "hyb" in mode:
        encT = nc.declare_dram_parameter("encT", [BPC, He - 256, S], F32R,
                                         isOutput=False)
        wh = nc.declare_dram_parameter("wh", [128, (KT - 2) * HT * 128], F32R,
                                       isOutput=False)
        encT8 = nc.declare_dram_parameter("encT8", [BPC, 128, 2, S], F8,
                                          isOutput=False)
        wh8 = nc.declare_dram_parameter("wh8", [128, HT, 2, 128], F8,
                                        isOutput=False)
    elif MM_DTYPE == "fp8dr":
        encT = nc.declare_dram_parameter("encT", [BPC, KT2, 128, 2, S], F8,
                                         isOutput=False)
        wh = nc.declare_dram_parameter("wh", [128, KT2 * HT, 2, 128], F8,
                                       isOutput=False)
        encT8 = wh8 = None
    else:
        encT = nc.declare_dram_parameter("encT", [BPC, He, S], _mm_dt(),
                                         isOutput=False)
        wh = nc.declare_dram_parameter("wh", [128, KT * HT * 128], _mm_dt(),
                                       isOutput=False)
    cb = nc.declare_dram_parameter("cb", [128, BPC * HT], F32, isOutput=False)
    vdt = F32 if "dvesc" in mode else _mm_dt()
    vw = nc.declare_dram_parameter("vw", [128, HT], vdt, isOutput=False)
    onesp = nc.declare_dram_parameter("ones", [128, 1], F32R, isOutput=False)
    out = nc.declare_dram_parameter("out", [BPC, S], F32, isOutput=True)

    with tile.TileContext(nc) as tc:
        with (
            tc.tile_pool(name="consts", bufs=1) as consts,
            tc.tile_pool(name="enc", bufs=24) as enc_pool,
            tc.tile_pool(name="th", bufs=10) as th_pool,
            tc.tile_pool(name="soft", bufs=2) as soft_pool,
            tc.tile_pool(name="ep", bufs=ep_bufs, space="PSUM") as ep_pool,
            tc.tile_pool(name="sc", bufs=2, space="PSUM") as sc_pool,
        ):
            lazy_w = n_loop == 1 and MM_DTYPE != "fp8dr" and "hyb" not in mode
            w8_sb = None
            if "hyb" in mode:
                w8_sb = consts.tile([128, HT, 2, 128], F8)
                nc.sync.dma_start(out=w8_sb, in_=wh8[:])
            if "hyb" in mode:
                w_sb = consts.tile([128, (KT - 2) * HT * 128], F32R)
                nc.sync.dma_start(out=w_sb, in_=wh[:])
            elif MM_DTYPE == "fp8dr":
                w_sb = consts.tile([128, KT2 * HT, 2, 128], F8)
                for k2 in range(KT2):
                    nc.sync.dma_start(out=w_sb[:, k2 * HT:(k2 + 1) * HT, :, :],
                                      in_=wh[:, k2 * HT:(k2 + 1) * HT, :, :])
            else:
                w_sb = consts.tile([128, KT * HT * 128], _mm_dt())
                if not lazy_w:
                    ck = HT * 128
                    for k in range(KT):
                        nc.sync.dma_start(out=w_sb[:, k * ck:(k + 1) * ck],
                                          in_=wh[:, k * ck:(k + 1) * ck])
            v_sb = consts.tile([128, HT], vdt)
            nc.sync.dma_start(out=v_sb, in_=vw[:])
            ones_sb = consts.tile([128, 1], F32R)
            nc.sync.dma_start(out=ones_sb, in_=onesp[:])
            c_sb = consts.tile([128, BPC * HT], F32)
            nc.sync.dma_start(out=c_sb, in_=cb[:])

            pools = (enc_pool, th_pool, soft_pool, ep_pool, sc_pool)
            et_shared = None
            if "compute" in mode:
                et_shared = []
                for k in range(KT):
                    t = consts.tile([128, SB], _mm_dt(), tag=f"etc{k}")
                    nc.sync.dma_start(out=t, in_=encT[0, k * 128:(k + 1) * 128, 0:SB])
                    et_shared.append(t)
            params = (encT, out, w_sb, v_sb, c_sb, ones_sb, et_shared,
                      wh, lazy_w)
            if "hyb" in mode:
                params = params + (encT8, w8_sb)
            if n_loop == 1:
                _emit_body(nc, pools, params, batches, mode)
            else:
                with tc.For_i(0, n_loop, 1):
                    _emit_body(nc, pools, params, batches, mode)
    nc.compile()
    _NC_CACHE[key] = nc
    return nc


def _np_mm_dt():
    if MM_DTYPE == "bf16":
        import ml_dtypes
        return ml_dtypes.bfloat16
    if MM_DTYPE == "fp8dr":
        import ml_dtypes
        return ml_dtypes.float8_e4m3
    return np.float32


def prepare_in_maps(hidden, encoder_outputs, W_attn, b_attn, v_w,
                    hyb=False):
    if MODE.startswith("dr"):
        return prepare_in_maps_dr(hidden, encoder_outputs, W_attn, b_attn,
                                  v_w)
    mmdt = _np_mm_dt()
    hidden = np.ascontiguousarray(np.asarray(hidden, dtype=np.float32))
    enc = np.asarray(encoder_outputs, dtype=np.float32)
    W_attn = np.asarray(W_attn, dtype=np.float32)
    b_attn = np.asarray(b_attn, dtype=np.float32)
    v_w = np.asarray(v_w, dtype=np.float32)

    h = hidden[-1]                      # [B, He]
    W_h = W_attn[:He]                   # [He, Hd]
    W_e = W_attn[He:]                   # [He, Hd]
    c = (h @ W_h + b_attn).astype(np.float32)   # [B, Hd]

    wh8 = None
    if hyb:
        import ml_dtypes
        f8 = ml_dtypes.float8_e4m3
        # fp8 part: He[0:256]; wh8[p, hd, s, m] = W_e[s*128+p, hd*128+m]
        wh8 = np.ascontiguousarray(
            W_e[:256].reshape(2, 128, HT, 128).transpose(1, 2, 0, 3)
            .reshape(128, HT, 2, 128).astype(f8))
        wh = np.ascontiguousarray(
            W_e[256:].reshape(KT - 2, 128, HT, 128).transpose(1, 0, 2, 3)
            .reshape(128, -1).astype(np.float32))
    elif MM_DTYPE == "fp8dr":
        # wh[p, k2*HT+hd, s, m] = W_e[k2*256 + s*128 + p, hd*128+m]
        wh = np.ascontiguousarray(
            W_e.reshape(KT2, 2, 128, HT, 128).transpose(2, 0, 3, 1, 4)
            .reshape(128, KT2 * HT, 2, 128).astype(mmdt))
    else:
        # wh[p, (k*HT+hd)*128+m] = W_e[k*128+p, hd*128+m]
        wh = np.ascontiguousarray(
            W_e.reshape(KT, 128, HT, 128).transpose(1, 0, 2, 3).reshape(128, -1)
            .astype(mmdt))
    # vw[p, hd] = v_w[hd*128+p]
    vw_dt = np.float32 if (VW_F32 or MM_DTYPE == "fp8dr") else mmdt
    vw = np.ascontiguousarray(v_w.reshape(HT, 128).T.astype(vw_dt))

    in_maps = []
    for ci in range(N_CORES):
        bsl = slice(ci * BPC, (ci + 1) * BPC)
        encT8 = None
        if hyb:
            import ml_dtypes
            f8 = ml_dtypes.float8_e4m3
            # encT8[b, p, s, n] = enc[b, n, s*128 + p] for He[0:256]
            encT8 = np.ascontiguousarray(
                enc[bsl, :, :256].reshape(BPC, S, 2, 128)
                .transpose(0, 3, 2, 1).astype(f8))
            encT = np.ascontiguousarray(
                enc[bsl, :, 256:].transpose(0, 2, 1).astype(np.float32))
        elif MM_DTYPE == "fp8dr":
            # encT[b, k2, p, s, n] = enc[b, n, k2*256 + s*128 + p]
            encT = np.ascontiguousarray(
                enc[bsl].reshape(BPC, S, KT2, 2, 128)
                .transpose(0, 2, 4, 3, 1).astype(mmdt))
        else:
            encT = np.ascontiguousarray(
                enc[bsl].transpose(0, 2, 1).astype(mmdt))  # [BPC, He, S]
        cb = np.ascontiguousarray(
            c[bsl].reshape(BPC, HT, 128).transpose(2, 0, 1).reshape(128, -1))
        m = {"encT": encT, "wh": wh, "cb": cb, "vw": vw,
             "ones": np.ones((128, 1), np.float32)}
        if hyb:
            m["encT8"] = encT8
            m["wh8"] = wh8
        in_maps.append(m)
    return in_maps


def kernel(hidden, encoder_outputs, W_attn, b_attn, v_w):
    nc = build_nc()
    in_maps = prepare_in_maps(hidden, encoder_outputs, W_attn, b_attn, v_w)
    res = run_bass_kernel_spmd(nc, in_maps, core_ids=list(range(N_CORES)))
    return np.concatenate([res.results[i]["out"] for i in range(N_CORES)],
                          axis=0)



# revision 32
# speedup vs baseline: 1.0182x; 1.0182x over previous
"""Bahdanau additive attention scores on 8 TRN2 NeuronCores.

reference:
    h = hidden[-1]                                   # [B, He]
    e_proj = enc @ W_e;  h_proj = h @ W_h            # W_attn = [W_h; W_e]
    scores = tanh(h_proj[:,None,:] + e_proj + b) @ v # [B, S]
    out = softmax(scores, axis=1)

Graded mode "dr1024" (MODE below): the whole GEMM runs as fp8e4m3 with
perf_mode=DoubleRow -- K=256 channels per instruction, 2 MACs/cell/cycle
-- for 1024 DR matmuls/core vs 2048 f32r ones. Measured 307-315 us vs
544-582 us for the f32r baseline (~1.8x), rel err 1.780e-2 vs the 2e-2
gate. The error is bit-deterministic (verified identical across
independent compiles+runs: fixed inputs, fixed instruction order, fixed
HW numerics), so the 11% margin is real, not statistical.

fp8 specifics:
  - W_e pre-scaled x1024 (pow2) before the e4m3 cast so sigma=0.022
    weights clear the e4m3 subnormal zone (min normal 2^-6); descale is
    folded into the tanh ACT as scale=1/1024. Direct-cast err 2.1e-2 ->
    scaled 1.9e-2 (offline study, exact emulation).
  - enc cast e4m3 unscaled (sigma=1 is already well-placed; scaling
    measured no-op).
  - Error floor is the e4m3 3-bit mantissa; DR's internal e6m3 pair
    format means no fp8 variant beats it. Hybrid f32r/fp8 modes
    ("dr768": 1.555e-2 @ 374us, "dr512": ~1.3e-2) trade time for margin.
  - s-blocks processed in groups of 3 so each DR weight tile is reused
    across 3 consecutive matmuls (amortizes DR's 256-col LDWEIGHTS);
    measured worth ~1% (_ki A/B) -- LDW is almost fully hidden anyway.
  - PE-bound end to end: GEMM-only probe (_pe) is 297.4us vs 306.9 full,
    and the delta is almost exactly the 32 score-reduction matmuls.
    Non-PE exposure ~1.3us total.
  - "_sg" idealized-stream probe == production: PSUM group cycling, DMA
    deps and ACT interleave cost ~zero. Session spread 275.6-316.3us
    (median ~302) is fleet clock (~2.0-2.2 GHz effective), not kernel.
  - "dr1024_tt" (transposed softmax tail via dma_start_transpose,
    eliminating the 32 sc matmuls): numerically correct (1.780e-2) but
    508us -- 128 XBAR transposes/iter cost ~200us of DMA-queue time to
    save 8us of PE. Kept as a mode for the record; do not ship.

Previous f32r strategy (modes "full_dvesc" etc., kept below and still
selectable): pure data-parallel over batch (B=32 -> 4 per core), zero
collectives. Host-side prep (free, off the HW critical path):
  - c = h @ W_h + b_attn  folded into a per-(batch, hd-tile) bias vector
  - encoder shard pre-transposed to [b, He, S] so the contraction dim He
    lands on SBUF partitions with no on-device transposes
  - W_e pre-tiled to [128, (k, hd, m)] so each [K=128, M=128] lhsT tile is
    a contiguous slice

Device program per core (TileContext), per (batch, 512-col s-block):
  e_projT[hd] = sum_k W_e[k,hd].T @ encT[k]   8x8 f32r matmuls -> PSUM
                (f32r = tf32-like: 1 col/cycle vs 4 for fp32; measured
                 ~0.57 ns/col on this fleet, rel err ~1e-4)
  th[hd]  = tanh(e_projT[hd] + c[b,hd])       ScalarE, PSUM->SBUF, fused bias
  acc    += th[hd] * v[hd]                    VectorE scalar_tensor_tensor
  scores  = ones.T @ acc                      1 matmul: partition-reduce
  exp_row[s-block], partial = Exp(scores)     ScalarE with accum_out
The scores reduction + Exp for block i are deferred into block i+1's
matmul stream so PE never waits on ScalarE/VectorE results.
Per batch: total = sum(partials); the final scaling runs on ScalarE
(fused scale=1/total) in two chunks so the first chunk's output DMA
overlaps the second chunk's scaling.

Softmax skips the max-subtraction: scores are ~N(0, 0.65), |max| < ~4
over 128K samples, exp() is comfortably within f32 range.

Measured: ~490 us per invocation on a quiet chip, ~590 us under fleet
load (per-matmul cost is sequencer-bound ~230 ns quiet / stream-bound
~300 ns busy; 2080 matmuls is the information-theoretic minimum at the
K<=128, N<=512 instruction caps). ~70 TF/s/core effective on the
275 GFLOP GEMM. rel err 9.6e-5.

Rejected-but-measured alternatives (kept as modes for the record):
  - fp8e4 DoubleRow whole-GEMM ("fp8dr"): 1024 instructions, ~1.6x, but
    rel err 1.95e-2 sits on the 2e-2 gate.
  - hybrid He[0:256] fp8-DR + He[256:1024] f32r ("hyb"): 1792
    instructions, ~5-10% faster, rel err 1.00e-2 -- spends half the
    error budget for <10% speed; not worth it.
  - bf16, interleaved accumulation groups, single-group, PSUM buffer
    counts, weight reuse: all within noise of the f32r baseline.

build_nc(n_loop=N) wraps the body in an in-NEFF For_i loop -- used by
test.py to amortize the ~80 ms axon-tunnel dispatch cost when timing.
The graded path is build_nc() defaults.
"""

import os

import numpy as np

import concourse.mybir as mybir
import concourse.tile as tile
from concourse import bacc
from concourse.bass_utils import run_bass_kernel_spmd

N_CORES = 8
L, B, S, He, Hd = 2, 32, 4096, 1024, 1024
BPC = B // N_CORES  # batches per core
KT = He // 128      # contraction tiles
HT = Hd // 128      # hd tiles
SB = 512            # s-block (matmul moving free dim)
NSB = S // SB
F32 = mybir.dt.float32
F32R = mybir.dt.float32r
BF16 = mybir.dt.bfloat16
F16 = mybir.dt.float16

# matmul-input dtype for the big GEMM:
#   "f32r"  - tf32-like, 1 col/cycle
#   "bf16"  - 1 col/cycle
#   "fp8dr" - float8e4 with DoubleRow: K=256 per instruction, 2 multiplies/cycle
MM_DTYPE = "f32r"
# v-dot on DVE ("dvesc" mode) keeps v in f32
VW_F32 = True
F8 = mybir.dt.float8e4
KT2 = KT // 2  # 256-deep contraction tiles for DoubleRow

# --- dr-hybrid mode ("dr<k8>[_ki][_pe]"): first k8 channels fp8e4+DoubleRow,
# rest f32r, s-blocks processed in groups so DR weight tiles are reused
# across consecutive matmuls (amortizes the 2x LDWEIGHTS cost of DR).
#   _ki: k-inner loop order (weights change every matmul) -- A/B probe
#   _pe: emit only the GEMM (no ACT/DVE/softmax) -- PE-bound probe
MODE = os.environ.get("K_MODE", "dr1024")
W_SCALE = 1024.0  # pow2; keeps fp8(W) out of the subnormal zone
DR_GROUPS = ((0, 1, 2), (3, 4, 5), (6, 7))

_NC_CACHE = {}


def _dr_k8(mode):
    return int(mode[2:].split("_")[0])


def _mm_dt():
    return BF16 if MM_DTYPE == "bf16" else F32R


def _emit_body(nc, pools, params, batches=None, mode="full"):
    AFT = mybir.ActivationFunctionType
    enc_pool, th_pool, soft_pool, ep_pool, sc_pool = pools
    encT, out, w_sb, v_sb, c_sb, ones_sb, et_shared, wh, lazy_w = params[:9]
    hyb = "hyb" in mode
    if hyb:
        encT8, w8_sb = params[9:]
    batches = list(range(BPC)) if batches is None else batches
    fp8 = MM_DTYPE == "fp8dr"
    th_dt = F32 if "dvesc" in mode else _mm_dt()

    # flat list of (batch, s-block); scores finalization for block i is
    # deferred into block i+1 so PE never waits on ACT/DVE results
    blocks = [(b, isb) for b in batches for isb in range(NSB)]
    soft = {}    # b -> (exp_row, parts)
    deferred = None  # (b, isb, sc_or_acc, ths)

    def finish_block(dfr):
        b, isb, acc, ths = dfr
        exp_row, parts = soft[b]
        if "dvesc" in mode:
            sc = sc_pool.tile([1, SB], F32, tag="sc")
            nc.tensor.matmul(sc, ones_sb, acc, start=True, stop=True)
        else:
            sc = sc_pool.tile([1, SB], F32, tag="sc")
            for hd in range(HT):
                nc.tensor.matmul(sc, v_sb[:, hd:hd + 1], ths[hd],
                                 start=(hd == 0), stop=(hd == HT - 1))
        nc.scalar.activation(
            exp_row[:, isb * SB:(isb + 1) * SB], sc, AFT.Exp,
            accum_out=parts[:, isb:isb + 1])
        if isb == NSB - 1:
            # batch done: softmax normalization + output
            tot = soft_pool.tile([1, 1], F32, tag="tot")
            nc.vector.tensor_reduce(tot, parts, axis=mybir.AxisListType.X,
                                    op=mybir.AluOpType.add)
            rinv = soft_pool.tile([1, 1], F32, tag="rinv")
            nc.vector.reciprocal(rinv, tot)
            # scale on ScalarE (1.2 GHz vs DVE 0.96 single-lane), in two
            # chunks so the first chunk's output DMA overlaps the second
            # chunk's scaling -- trims the exposed final-batch tail
            half = S // 2
            for c2 in range(2):
                oc = soft_pool.tile([1, half], F32, tag="oc", bufs=4,
                                    name=f"oc_{b}_{c2}")
                nc.scalar.activation(oc, exp_row[:, c2 * half:(c2 + 1) * half],
                                     AFT.Copy, scale=rinv)
                nc.sync.dma_start(out=out[b:b + 1, c2 * half:(c2 + 1) * half],
                                  in_=oc)
            del soft[b]

    for b, isb in blocks:
        if b not in soft:
            soft[b] = (soft_pool.tile([1, S], F32, tag="exp_row",
                                      name=f"exp_row_{b}"),
                       soft_pool.tile([1, NSB], F32, tag="parts",
                                      name=f"parts_{b}"))
        if "compute" in mode:
            et = et_shared
        elif hyb:
            et8 = enc_pool.tile([128, 2, SB], F8, tag="et8")
            nc.sync.dma_start(
                out=et8, in_=encT8[b, :, :, isb * SB:(isb + 1) * SB])
            et = []
            for k in range(KT - 2):
                t = enc_pool.tile([128, SB], F32R, tag="et")
                nc.sync.dma_start(
                    out=t,
                    in_=encT[b, k * 128:(k + 1) * 128, isb * SB:(isb + 1) * SB])
                et.append(t)
        elif fp8:
            et = []
            for k2 in range(KT2):
                t = enc_pool.tile([128, 2, SB], F8, tag="et")
                nc.sync.dma_start(
                    out=t, in_=encT[b, k2, :, :, isb * SB:(isb + 1) * SB])
                et.append(t)
        elif "wet" in mode:
            # wide et: one [128, 2*SB] tile per k covers two s-blocks --
            # halves DMA count and first-use sem waits
            first = (b, isb) == blocks[0]
            if isb % 2 == 0:
                etw = []
                for k in range(KT):
                    t = enc_pool.tile([128, 2 * SB], _mm_dt(), tag="etw",
                                      bufs=12, name=f"etw{k}")
                    nc.sync.dma_start(
                        out=t,
                        in_=encT[b, k * 128:(k + 1) * 128,
                                 isb * SB:(isb + 2) * SB])
                    etw.append(t)
                    if lazy_w and first:
                        ck = HT * 128
                        nc.sync.dma_start(out=w_sb[:, k * ck:(k + 1) * ck],
                                          in_=wh[:, k * ck:(k + 1) * ck])
                _emit_body.etw = etw
            off = (isb % 2) * SB
            et = [t[:, off:off + SB] for t in _emit_body.etw]
        else:
            first = (b, isb) == blocks[0]
            et = []
            for k in range(KT):
                t = enc_pool.tile([128, SB], _mm_dt(), tag="et")
                nc.sync.dma_start(
                    out=t,
                    in_=encT[b, k * 128:(k + 1) * 128, isb * SB:(isb + 1) * SB])
                et.append(t)
                if lazy_w and first:
                    ck = HT * 128
                    nc.sync.dma_start(out=w_sb[:, k * ck:(k + 1) * ck],
                                      in_=wh[:, k * ck:(k + 1) * ck])

        acc = None
        ths = []
        if "g1" in mode:
            first_blk = (b, isb) == blocks[0]
            last_blk = (b, isb) == blocks[-1]
            ep = ep_pool.tile([128, SB], F32, tag="ep1", bufs=1,
                              name="ep_g1")
            for hd in range(HT):
                for k in range(KT):
                    w_tile = w_sb[:, (k * HT + hd) * 128:(k * HT + hd + 1) * 128]
                    nc.tensor.matmul(
                        ep, w_tile, et[k],
                        start=(first_blk and hd == 0 and k == 0),
                        stop=(last_blk and hd == HT - 1 and k == KT - 1))
            if last_blk:
                probe = soft_pool.tile([128, 1], F32, tag="probe")
                nc.scalar.activation(probe, ep[:, 0:1], AFT.Copy)
            continue
        for hd in range(HT):
            ep = ep_pool.tile([128, SB], F32, tag="ep")
            if fp8:
                for k2 in range(KT2):
                    nc.tensor.matmul(
                        ep, w_sb[:, k2 * HT + hd, :, :], et[k2],
                        start=(k2 == 0), stop=(k2 == KT2 - 1),
                        perf_mode=mybir.MatmulPerfMode.DoubleRow)
            elif hyb:
                nc.tensor.matmul(ep, w8_sb[:, hd, :, :], et8,
                                 start=True, stop=False,
                                 perf_mode=mybir.MatmulPerfMode.DoubleRow)
                for k in range(KT - 2):
                    w_tile = w_sb[:, (k * HT + hd) * 128:(k * HT + hd + 1) * 128]
                    nc.tensor.matmul(ep, w_tile, et[k],
                                     start=False, stop=(k == KT - 3))
            else:
                kr = range(KT // 2) if "k4" in mode else range(KT)
                for k in kr:
                    w_tile = w_sb[:, (k * HT + hd) * 128:(k * HT + hd + 1) * 128]
                    nc.tensor.matmul(ep, w_tile, et[k],
                                     start=(k == 0),
                                     stop=(k == list(kr)[-1]))
            if hd == 1 and deferred is not None:
                # PE work for the previous block's scores goes here, long
                # after its inputs are ready
                finish_block(deferred)
                deferred = None
            if "noact" in mode:
                if hd == HT - 1:
                    probe = soft_pool.tile([128, 1], F32, tag="probe")
                    nc.scalar.activation(probe, ep[:, 0:1], AFT.Copy)
                continue
            th = th_pool.tile([128, SB], th_dt, tag="th")
            nc.scalar.activation(
                th, ep, AFT.Tanh, bias=c_sb[:, b * HT + hd: b * HT + hd + 1])
            ths.append(th)
            if "dvesc" in mode:
                if hd == 0:
                    acc = th_pool.tile([128, SB], F32, tag="acc", bufs=3)
                    nc.vector.tensor_scalar_mul(acc, th, v_sb[:, 0:1])
                else:
                    nc.vector.scalar_tensor_tensor(
                        acc, th, v_sb[:, hd:hd + 1], acc,
                        op0=mybir.AluOpType.mult, op1=mybir.AluOpType.add)
        if "noact" in mode:
            continue
        if "dvesc" in mode:
            acc8 = th_pool.tile([128, SB], F32R, tag="acc8", bufs=3)
            nc.scalar.activation(acc8, acc, AFT.Copy)
            acc = acc8
        deferred = (b, isb, acc if "dvesc" in mode else None, ths)
    if deferred is not None and "noact" not in mode:
        finish_block(deferred)


def _emit_dr_body(nc, pools, params, batches=None, mode="dr768"):
    AFT = mybir.ActivationFunctionType
    DR = mybir.MatmulPerfMode.DoubleRow
    enc_pool, th_pool, soft_pool, ep_pool, sc_pool = pools
    (enc8, encf, out, w8_sb, wf_sb, v_sb, ones_sb, c_sb, w8p, wfp,
     lazy_w, ones2_sb) = params
    batches = list(range(BPC)) if batches is None else batches
    k8 = _dr_k8(mode)
    kt2 = k8 // 256
    kf = (He - k8) // 128
    ki = "_ki" in mode
    probe_pe = "_pe" in mode
    tt = "_tt" in mode  # transposed softmax tail: no per-block sc matmuls
    ds = 1.0 / W_SCALE

    blocks = [(b, g) for b in batches for g in range(len(DR_GROUPS))]
    soft = {}
    finq = []  # deferred (b, isb, accT) score finishers / tt batch ends

    def finish_batch_tt():
        b = finq.pop(0)
        exp_t, parts_t = soft[b]
        # ones2 is [128,128]: every output partition gets the per-isb
        # cross-partition totals in one N=8 matmul
        tot8 = sc_pool.tile([128, NSB], F32, tag="tot8")
        nc.tensor.matmul(tot8, ones2_sb, parts_t, start=True, stop=True)
        tot = soft_pool.tile([128, 1], F32, tag="tot128")
        nc.vector.tensor_reduce(tot, tot8, axis=mybir.AxisListType.X,
                                op=mybir.AluOpType.add)
        rinv = soft_pool.tile([128, 1], F32, tag="rinv128")
        nc.vector.reciprocal(rinv, tot)
        oc = soft_pool.tile([128, S // 128], F32, tag="oct", bufs=4,
                            name=f"oct_{b}")
        nc.scalar.activation(oc, exp_t, mybir.ActivationFunctionType.Copy,
                             scale=rinv)
        nc.sync.dma_start(
            out=out[b:b + 1, :].rearrange("a (c p) -> (a p) c", p=128),
            in_=oc)
        del soft[b]

    def finish_one():
        b, isb, acc = finq.pop(0)
        exp_row, parts = soft[b]
        sc = sc_pool.tile([1, SB], F32, tag="sc")
        nc.tensor.matmul(sc, ones_sb, acc, start=True, stop=True)
        nc.scalar.activation(
            exp_row[:, isb * SB:(isb + 1) * SB], sc, AFT.Exp,
            accum_out=parts[:, isb:isb + 1])
        if isb == NSB - 1:
            tot = soft_pool.tile([1, 1], F32, tag="tot")
            nc.vector.tensor_reduce(tot, parts, axis=mybir.AxisListType.X,
                                    op=mybir.AluOpType.add)
            rinv = soft_pool.tile([1, 1], F32, tag="rinv")
            nc.vector.reciprocal(rinv, tot)
            half = S // 2
            for c2 in range(2):
                oc = soft_pool.tile([1, half], F32, tag="oc", bufs=4,
                                    name=f"oc_{b}_{c2}")
                nc.scalar.activation(oc, exp_row[:, c2 * half:(c2 + 1) * half],
                                     AFT.Copy, scale=rinv)
                nc.sync.dma_start(out=out[b:b + 1, c2 * half:(c2 + 1) * half],
                                  in_=oc)
            del soft[b]

    for b, g in blocks:
        isbs = DR_GROUPS[g]
        ni = len(isbs)
        s0 = isbs[0] * SB
        gw = ni * SB
        if b not in soft and not probe_pe:
            if tt:
                soft[b] = (soft_pool.tile([128, S // 128], F32, tag="exp_t",
                                          name=f"exp_t_{b}"),
                           soft_pool.tile([128, NSB], F32R, tag="parts_t",
                                          name=f"parts_t_{b}"))
            else:
                soft[b] = (soft_pool.tile([1, S], F32, tag="exp_row",
                                          name=f"exp_row_{b}"),
                           soft_pool.tile([1, NSB], F32, tag="parts",
                                          name=f"parts_{b}"))
        first = (b, g) == blocks[0]
        et8 = []
        for k2 in range(kt2):
            t = enc_pool.tile([128, 2, gw], F8, tag=f"et8_{k2}", bufs=3)
            nc.sync.dma_start(out=t, in_=enc8[b, k2, :, :, s0:s0 + gw])
            et8.append(t)
            if lazy_w and first:
                nc.sync.dma_start(
                    out=w8_sb[:, k2 * HT:(k2 + 1) * HT, :, :],
                    in_=w8p[:, k2 * HT:(k2 + 1) * HT, :, :])
        etf = []
        for k in range(kf):
            t = enc_pool.tile([128, gw], F32R, tag=f"etf_{k}", bufs=3)
            nc.sync.dma_start(out=t, in_=encf[b, k * 128:(k + 1) * 128,
                                             s0:s0 + gw])
            etf.append(t)
            if lazy_w and first:
                ck = HT * 128
                nc.sync.dma_start(out=wf_sb[:, k * ck:(k + 1) * ck],
                                  in_=wfp[:, k * ck:(k + 1) * ck])
        if "_sg" in mode:
            # pure-stream probe: one PSUM accumulation group, constant
            # weight+rhs, no ACT/DVE -- measures the intrinsic DR matmul
            # stream floor (PSUM cycling + dep-wait cost excluded)
            first_blk = (b, g) == blocks[0]
            last_blk = (b, g) == blocks[-1]
            epg = ep_pool.tile([128, SB], F32, tag="epg", bufs=1,
                               name="ep_sg")
            nmm_all = HT * ni * kt2
            mi = 0
            for hd in range(HT):
                for k2 in range(kt2):
                    for i in range(ni):
                        nc.tensor.matmul(
                            epg, w8_sb[:, 0, :, :], et8[0][:, :, 0:SB],
                            start=(first_blk and mi == 0),
                            stop=(last_blk and mi == nmm_all - 1),
                            perf_mode=DR)
                        mi += 1
            if last_blk:
                pr = soft_pool.tile([128, 1], F32, tag="probe")
                nc.scalar.activation(pr, epg[:, 0:1], AFT.Copy)
            continue
        accs = [None] * ni
        for hd in range(HT):
            eps = []
            for i in range(ni):
                ep = ep_pool.tile([128, SB], F32, tag="ep", name=f"ep{i}")
                eps.append(ep)
            nmm = kt2 + kf

            def mm(i, mi, k2=None, k=None):
                if k2 is not None:
                    if "_1w" in mode:  # probe: constant weight+rhs tiles
                        nc.tensor.matmul(
                            eps[i], w8_sb[:, 0, :, :], et8[0][:, :, 0:SB],
                            start=(mi == 0), stop=(mi == nmm - 1),
                            perf_mode=DR)
                        return
                    nc.tensor.matmul(
                        eps[i], w8_sb[:, k2 * HT + hd, :, :],
                        et8[k2][:, :, i * SB:(i + 1) * SB],
                        start=(mi == 0), stop=(mi == nmm - 1), perf_mode=DR)
                else:
                    nc.tensor.matmul(
                        eps[i],
                        wf_sb[:, (k * HT + hd) * 128:(k * HT + hd + 1) * 128],
                        etf[k][:, i * SB:(i + 1) * SB],
                        start=(mi == 0), stop=(mi == nmm - 1))

            if ki:  # weights swapped every matmul (A/B probe)
                for i in range(ni):
                    for mi, k2 in enumerate(range(kt2)):
                        mm(i, mi, k2=k2)
                    for mi, k in enumerate(range(kf)):
                        mm(i, kt2 + mi, k=k)
            else:   # weight tile reused across the group's s-blocks
                for mi, k2 in enumerate(range(kt2)):
                    for i in range(ni):
                        mm(i, mi, k2=k2)
                for mi, k in enumerate(range(kf)):
                    for i in range(ni):
                        mm(i, kt2 + mi, k=k)

            if probe_pe:
                if (b, g) == blocks[-1] and hd == HT - 1:
                    pr = soft_pool.tile([128, 1], F32, tag="probe")
                    nc.scalar.activation(pr, eps[-1][:, 0:1], AFT.Copy)
                continue
            if hd >= 1 and finq:
                finish_batch_tt() if tt else finish_one()
            for i in range(ni):
                th = th_pool.tile([128, SB], F32, tag="th")
                nc.scalar.activation(
                    th, eps[i], AFT.Tanh,
                    bias=c_sb[:, b * HT + hd:b * HT + hd + 1], scale=ds)
                if hd == 0:
                    acc = th_pool.tile([128, SB], F32 if tt else F32R,
                                       tag="acc", bufs=8, name=f"acc{i}")
                    accs[i] = acc
                    nc.vector.tensor_scalar_mul(accs[i], th, v_sb[:, 0:1])
                else:
                    nc.vector.scalar_tensor_tensor(
                        accs[i], th, v_sb[:, hd:hd + 1], accs[i],
                        op0=mybir.AluOpType.mult, op1=mybir.AluOpType.add)
        if probe_pe:
            continue
        if tt:
            exp_t, parts_t = soft[b]
            for i, isb in enumerate(isbs):
                a16 = th_pool.tile([128, SB], F16, tag="a16", bufs=4,
                                   name=f"a16_{i}")
                nc.scalar.activation(a16, accs[i],
                                     mybir.ActivationFunctionType.Copy)
                ttile = th_pool.tile([128, SB // 128, 128], F16, tag="ttile",
                                     bufs=4, name=f"ttile_{i}")
                for c in range(SB // 128):
                    nc.sync.dma_start_transpose(
                        ttile[:, c, :], a16[:, c * 128:(c + 1) * 128])
                st = th_pool.tile([128, SB // 128], F32, tag="st", bufs=4,
                                  name=f"st_{i}")
                with nc.allow_low_precision(
                        reason="fp16 transpose staging adds ~0.05% to "
                               "scores; fp32 reduce output"):
                    nc.vector.tensor_reduce(st, ttile,
                                            axis=mybir.AxisListType.X,
                                            op=mybir.AluOpType.add)
                with nc.allow_low_precision(
                        reason="per-block exp partials rounded to f32r for "
                               "the broadcast matmul; 2^-12 relative"):
                    nc.scalar.activation(
                        exp_t[:, isb * 4:(isb + 1) * 4], st,
                        mybir.ActivationFunctionType.Exp,
                        accum_out=parts_t[:, isb:isb + 1])
            if isbs[-1] == NSB - 1:
                finq.append(b)
        else:
            for i, isb in enumerate(isbs):
                finq.append((b, isb, accs[i]))
    while finq:
        finish_batch_tt() if tt else finish_one()


def _build_nc_dr(n_loop, batches, mode, ep_bufs):
    k8 = _dr_k8(mode)
    kt2 = k8 // 256
    kf = (He - k8) // 128
    nc = bacc.Bacc(trn_type="TRN2", target_bir_lowering=False, debug=False,
                   num_devices=N_CORES)
    enc8 = nc.declare_dram_parameter("enc8", [BPC, kt2, 128, 2, S], F8,
                                     isOutput=False)
    w8p = nc.declare_dram_parameter("w8", [128, kt2 * HT, 2, 128], F8,
                                    isOutput=False)
    encf = wfp = None
    if kf:
        encf = nc.declare_dram_parameter("encf", [BPC, kf * 128, S], F32R,
                                         isOutput=False)
        wfp = nc.declare_dram_parameter("wf", [128, kf * HT * 128], F32R,
                                        isOutput=False)
    cb = nc.declare_dram_parameter("cb", [128, BPC * HT], F32, isOutput=False)
    vw = nc.declare_dram_parameter("vw", [128, HT], F32, isOutput=False)
    onesp = nc.declare_dram_parameter("ones", [128, 1], F32R, isOutput=False)
    ones2p = nc.declare_dram_parameter("ones2", [128, 128], F32R,
                                       isOutput=False)
    out = nc.declare_dram_parameter("out", [BPC, S], F32, isOutput=True)

    with tile.TileContext(nc) as tc:
        with (
            tc.tile_pool(name="consts", bufs=1) as consts,
            tc.tile_pool(name="enc", bufs=2) as enc_pool,
            tc.tile_pool(name="th", bufs=10) as th_pool,
            tc.tile_pool(name="soft", bufs=2) as soft_pool,
            tc.tile_pool(name="ep", bufs=ep_bufs, space="PSUM") as ep_pool,
            tc.tile_pool(name="sc", bufs=2, space="PSUM") as sc_pool,
        ):
            lazy_w = n_loop == 1
            w8_sb = consts.tile([128, kt2 * HT, 2, 128], F8)
            if not lazy_w:
                nc.sync.dma_start(out=w8_sb, in_=w8p[:])
            wf_sb = None
            if kf:
                wf_sb = consts.tile([128, kf * HT * 128], F32R)
                if not lazy_w:
                    ck = HT * 128
                    for k in range(kf):
                        nc.sync.dma_start(out=wf_sb[:, k * ck:(k + 1) * ck],
                                          in_=wfp[:, k * ck:(k + 1) * ck])
            v_sb = consts.tile([128, HT], F32)
            nc.sync.dma_start(out=v_sb, in_=vw[:])
            ones_sb = consts.tile([128, 1], F32R)
            nc.sync.dma_start(out=ones_sb, in_=onesp[:])
            ones2_sb = consts.tile([128, 128], F32R)
            nc.sync.dma_start(out=ones2_sb, in_=ones2p[:])
            c_sb = consts.tile([128, BPC * HT], F32)
            nc.sync.dma_start(out=c_sb, in_=cb[:])

            pools = (enc_pool, th_pool, soft_pool, ep_pool, sc_pool)
            params = (enc8, encf, out, w8_sb, wf_sb, v_sb, ones_sb, c_sb,
                      w8p, wfp, lazy_w, ones2_sb)
            if n_loop == 1:
                _emit_dr_body(nc, pools, params, batches, mode)
            else:
                with tc.For_i(0, n_loop, 1):
                    _emit_dr_body(nc, pools, params, batches, mode)
    nc.compile()
    return nc


def prepare_in_maps_dr(hidden, encoder_outputs, W_attn, b_attn, v_w,
                       mode=None):
    import ml_dtypes
    f8 = ml_dtypes.float8_e4m3
    mode = MODE if mode is None else mode
    k8 = _dr_k8(mode)
    kt2 = k8 // 256
    kf = (He - k8) // 128
    hidden = np.ascontiguousarray(np.asarray(hidden, dtype=np.float32))
    enc = np.asarray(encoder_outputs, dtype=np.float32)
    W_attn = np.asarray(W_attn, dtype=np.float32)
    b_attn = np.asarray(b_attn, dtype=np.float32)
    v_w = np.asarray(v_w, dtype=np.float32)

    h = hidden[-1]
    W_h = W_attn[:He]
    W_e = W_attn[He:]
    c = (h @ W_h + b_attn).astype(np.float32)   # [B, Hd]

    # w8[p, k2*HT+hd, j, m] = SW * W_e[k2*256 + j*128 + p, hd*128 + m]
    w8 = np.ascontiguousarray(
        (W_e[:k8] * W_SCALE).reshape(kt2, 2, 128, HT, 128)
        .transpose(2, 0, 3, 1, 4).reshape(128, kt2 * HT, 2, 128).astype(f8))
    wf = None
    if kf:
        # wf[p, (k*HT+hd)*128+m] = SW * W_e[k8 + k*128 + p, hd*128 + m]
        wf = np.ascontiguousarray(
            (W_e[k8:] * W_SCALE).reshape(kf, 128, HT, 128)
            .transpose(1, 0, 2, 3).reshape(128, -1).astype(np.float32))
    vw = np.ascontiguousarray(v_w.reshape(HT, 128).T.astype(np.float32))

    in_maps = []
    for ci in range(N_CORES):
        bsl = slice(ci * BPC, (ci + 1) * BPC)
        # enc8[b, k2, p, j, s] = enc[b, s, k2*256 + j*128 + p]
        enc8 = np.ascontiguousarray(
            enc[bsl, :, :k8].reshape(BPC, S, kt2, 2, 128)
            .transpose(0, 2, 4, 3, 1).astype(f8))
        cbm = np.ascontiguousarray(
            c[bsl].reshape(BPC, HT, 128).transpose(2, 0, 1).reshape(128, -1))
        m = {"enc8": enc8, "w8": w8, "cb": cbm, "vw": vw,
             "ones": np.ones((128, 1), np.float32),
             "ones2": np.ones((128, 128), np.float32)}
        if kf:
            m["encf"] = np.ascontiguousarray(
                enc[bsl, :, k8:].transpose(0, 2, 1).astype(np.float32))
            m["wf"] = wf
        in_maps.append(m)
    return in_maps


def build_nc(n_loop=1, batches=None, mode=None, ep_bufs=None):
    mode = MODE if mode is None else mode
    if ep_bufs is None:
        ep_bufs = 6 if mode.startswith("dr") else 4
    key = (MM_DTYPE, n_loop, tuple(batches) if batches else None, mode, ep_bufs)
    if key in _NC_CACHE:
        return _NC_CACHE[key]
    if mode.startswith("dr"):
        nc = _build_nc_dr(n_loop, batches, mode, ep_bufs)
        _NC_CACHE[key] = nc
        return nc
    return _build_nc_orig(n_loop, batches, mode, ep_bufs)


def _build_nc_orig(n_loop=1, batches=None, mode="full_dvesc", ep_bufs=4):
    key = (MM_DTYPE, n_loop, tuple(batches) if batches else None, mode, ep_bufs)
    if key in _NC_CACHE:
        return _NC_CACHE[key]
    nc = bacc.Bacc(trn_type="TRN2", target_bir_lowering=False, debug=False,
                   num_devices=N_CORES)
    if "hyb" in mode:
        encT = nc.declare_dram_parameter("encT", [BPC, He - 256, S], F32R,
                                         isOutput=False)
        wh = nc.declare_dram_parameter("wh", [128, (KT - 2) * HT * 128], F32R,
                                       isOutput=False)
        encT8 = nc.declare_dram_parameter("encT8", [BPC, 128, 2, S], F8,
                                          isOutput=False)
        wh8 = nc.declare_dram_parameter("wh8", [128, HT, 2, 128], F8,
                                        isOutput=False)
    elif MM_DTYPE == "fp8dr":
        encT = nc.declare_dram_parameter("encT", [BPC, KT2, 128, 2, S], F8,
                                         isOutput=False)
        wh = nc.declare_dram_parameter("wh", [128, KT2 * HT, 2, 128], F8,
                                       isOutput=False)
        encT8 = wh8 = None
    else:
        encT = nc.declare_dram_parameter("encT", [BPC, He, S], _mm_dt(),
                                         isOutput=False)
        wh = nc.declare_dram_parameter("wh", [128, KT * HT * 128], _mm_dt(),
                                       isOutput=False)
    cb = nc.declare_dram_parameter("cb", [128, BPC * HT], F32, isOutput=False)
    vdt = F32 if "dvesc" in mode else _mm_dt()
    vw = nc.declare_dram_parameter("vw", [128, HT], vdt, isOutput=False)
    onesp = nc.declare_dram_parameter("ones", [128, 1], F32R, isOutput=False)
    out = nc.declare_dram_parameter("out", [BPC, S], F32, isOutput=True)

    with tile.TileContext(nc) as tc:
        with (
            tc.tile_pool(name="consts", bufs=1) as consts,
            tc.tile_pool(name="enc", bufs=24) as enc_pool,
            tc.tile_pool(name="th", bufs=10) as th_pool,
            tc.tile_pool(name="soft", bufs=2) as soft_pool,
            tc.tile_pool(name="ep", bufs=ep_bufs, space="PSUM") as ep_pool,
            tc.tile_pool(name="sc", bufs=2, space="PSUM") as sc_pool,
        ):
            lazy_w = n_loop == 1 and MM_DTYPE != "fp8dr" and "hyb" not in mode
            w8_sb = None
            if "hyb" in mode:
                w8_sb = consts.tile([128, HT, 2, 128], F8)
                nc.sync.dma_start(out=w8_sb, in_=wh8[:])
            if "hyb" in mode:
                w_sb = consts.tile([128, (KT - 2) * HT * 128], F32R)
                nc.sync.dma_start(out=w_sb, in_=wh[:])
            elif MM_DTYPE == "fp8dr":
                w_sb = consts.tile([128, KT2 * HT, 2, 128], F8)
                for k2 in range(KT2):
                    nc.sync.dma_start(out=w_sb[:, k2 * HT:(k2 + 1) * HT, :, :],
                                      in_=wh[:, k2 * HT:(k2 + 1) * HT, :, :])
            else:
                w_sb = consts.tile([128, KT * HT * 128], _mm_dt())
                if not lazy_w:
                    ck = HT * 128
                    for k in range(KT):
                        nc.sync.dma_start(out=w_sb[:, k * ck:(k + 1) * ck],
                                          in_=wh[:, k * ck:(k + 1) * ck])
            v_sb = consts.tile([128, HT], vdt)
            nc.sync.dma_start(out=v_sb, in_=vw[:])
            ones_sb = consts.tile([128, 1], F32R)
            nc.sync.dma_start(out=ones_sb, in_=onesp[:])
            c_sb = consts.tile([128, BPC * HT], F32)
            nc.sync.dma_start(out=c_sb, in_=cb[:])

            pools = (enc_pool, th_pool, soft_pool, ep_pool, sc_pool)
            et_shared = None
            if "compute" in mode:
                et_shared = []
                for k in range(KT):
                    t = consts.tile([128, SB], _mm_dt(), tag=f"etc{k}")
                    nc.sync.dma_start(out=t, in_=encT[0, k * 128:(k + 1) * 128, 0:SB])
                    et_shared.append(t)
            params = (encT, out, w_sb, v_sb, c_sb, ones_sb, et_shared,
                      wh, lazy_w)
            if "hyb" in mode:
                params = params + (encT8, w8_sb)
            if n_loop == 1:
                _emit_body(nc, pools, params, batches, mode)
            else:
                with tc.For_i(0, n_loop, 1):
                    _emit_body(nc, pools, params, batches, mode)
    nc.compile()
    _NC_CACHE[key] = nc
    return nc


def _np_mm_dt():
    if MM_DTYPE == "bf16":
        import ml_dtypes
        return ml_dtypes.bfloat16
    if MM_DTYPE == "fp8dr":
        import ml_dtypes
        return ml_dtypes.float8_e4m3
    return np.float32


def prepare_in_maps(hidden, encoder_outputs, W_attn, b_attn, v_w,
                    hyb=False):
    if MODE.startswith("dr"):
        return prepare_in_maps_dr(hidden, encoder_outputs, W_attn, b_attn,
                                  v_w)
    mmdt = _np_mm_dt()
    hidden = np.ascontiguousarray(np.asarray(hidden, dtype=np.float32))
    enc = np.asarray(encoder_outputs, dtype=np.float32)
    W_attn = np.asarray(W_attn, dtype=np.float32)
    b_attn = np.asarray(b_attn, dtype=np.float32)
    v_w = np.asarray(v_w, dtype=np.float32)

    h = hidden[-1]                      # [B, He]
    W_h = W_attn[:He]                   # [He, Hd]
    W_e = W_attn[He:]                   # [He, Hd]
    c = (h @ W_h + b_attn).astype(np.float32)   # [B, Hd]

    wh8 = None
    if hyb:
        import ml_dtypes
        f8 = ml_dtypes.float8_e4m3
        # fp8 part: He[0:256]; wh8[p, hd, s, m] = W_e[s*128+p, hd*128+m]
        wh8 = np.ascontiguousarray(
            W_e[:256].reshape(2, 128, HT, 128).transpose(1, 2, 0, 3)
            .reshape(128, HT, 2, 128).astype(f8))
        wh = np.ascontiguousarray(
            W_e[256:].reshape(KT - 2, 128, HT, 128).transpose(1, 0, 2, 3)
            .reshape(128, -1).astype(np.float32))
    elif MM_DTYPE == "fp8dr":
        # wh[p, k2*HT+hd, s, m] = W_e[k2*256 + s*128 + p, hd*128+m]
        wh = np.ascontiguousarray(
            W_e.reshape(KT2, 2, 128, HT, 128).transpose(2, 0, 3, 1, 4)
            .reshape(128, KT2 * HT, 2, 128).astype(mmdt))
    else:
        # wh[p, (k*HT+hd)*128+m] = W_e[k*128+p, hd*128+m]
        wh = np.ascontiguousarray(
            W_e.reshape(KT, 128, HT, 128).transpose(1, 0, 2, 3).reshape(128, -1)
            .astype(mmdt))
    # vw[p, hd] = v_w[hd*128+p]
    vw_dt = np.float32 if (VW_F32 or MM_DTYPE == "fp8dr") else mmdt
    vw = np.ascontiguousarray(v_w.reshape(HT, 128).T.astype(vw_dt))

    in_maps = []
    for ci in range(N_CORES):
        bsl = slice(ci * BPC, (ci + 1) * BPC)
        encT8 = None
        if hyb:
            import ml_dtypes
            f8 = ml_dtypes.float8_e4m3
            # encT8[b, p, s, n] = enc[b, n, s*128 + p] for He[0:256]
            encT8 = np.ascontiguousarray(
                enc[bsl, :, :256].reshape(BPC, S, 2, 128)
                .transpose(0, 3, 2, 1).astype(f8))
            encT = np.ascontiguousarray(
                enc[bsl, :, 256:].transpose(0, 2, 1).astype(np.float32))
        elif MM_DTYPE == "fp8dr":
            # encT[b, k2, p, s, n] = enc[b, n, k2*256 + s*128 + p]
            encT = np.ascontiguousarray(
                enc[bsl].reshape(BPC, S, KT2, 2, 128)
                .transpose(0, 2, 4, 3, 1).astype(mmdt))
        else:
            encT = np.ascontiguousarray(
                enc[bsl].transpose(0, 2, 1).astype(mmdt))  # [BPC, He, S]
        cb = np.ascontiguousarray(
            c[bsl].reshape(BPC, HT, 128).transpose(2, 0, 1).reshape(128, -1))
        m = {"encT": encT, "wh": wh, "cb": cb, "vw": vw,
             "ones": np.ones((128, 1), np.float32)}
        if hyb:
            m["encT8"] = encT8
            m["wh8"] = wh8
        in_maps.append(m)
    return in_maps


def kernel(hidden, encoder_outputs, W_attn, b_attn, v_w):
    nc = build_nc()
    in_maps = prepare_in_maps(hidden, encoder_outputs, W_attn, b_attn, v_w)
    res = run_bass_kernel_spmd(nc, in_maps, core_ids=list(range(N_CORES)))
    return np.concatenate([res.results[i]["out"] for i in range(N_CORES)],
                          axis=0)



# revision 34
# speedup vs baseline: 1.0598x; 1.0409x over previous
"""Bahdanau additive attention scores on 8 TRN2 NeuronCores.

reference:
    h = hidden[-1]                                   # [B, He]
    e_proj = enc @ W_e;  h_proj = h @ W_h            # W_attn = [W_h; W_e]
    scores = tanh(h_proj[:,None,:] + e_proj + b) @ v # [B, S]
    out = softmax(scores, axis=1)

Graded mode "dr1024" (MODE below): the whole GEMM runs as fp8e4m3 with
perf_mode=DoubleRow -- K=256 channels per instruction, 2 MACs/cell/cycle
-- for 1024 DR matmuls/core vs 2048 f32r ones. Measured 307-315 us vs
544-582 us for the f32r baseline (~1.8x), rel err 1.780e-2 vs the 2e-2
gate. The error is bit-deterministic (verified identical across
independent compiles+runs: fixed inputs, fixed instruction order, fixed
HW numerics), so the 11% margin is real, not statistical.

fp8 specifics:
  - W_e pre-scaled x1024 (pow2) before the e4m3 cast so sigma=0.022
    weights clear the e4m3 subnormal zone (min normal 2^-6); descale is
    folded into the tanh ACT as scale=1/1024. Direct-cast err 2.1e-2 ->
    scaled 1.9e-2 (offline study, exact emulation).
  - enc cast e4m3 unscaled (sigma=1 is already well-placed; scaling
    measured no-op).
  - Error floor is the e4m3 3-bit mantissa; DR's internal e6m3 pair
    format means no fp8 variant beats it. Hybrid f32r/fp8 modes
    ("dr768": 1.555e-2 @ 374us, "dr512": ~1.3e-2) trade time for margin.
  - s-blocks processed in groups of 3 so each DR weight tile is reused
    across 3 consecutive matmuls (amortizes DR's 256-col LDWEIGHTS);
    measured worth ~1% (_ki A/B) -- LDW is almost fully hidden anyway.
  - PE-bound end to end: GEMM-only probe (_pe) is 297.4us vs 306.9 full,
    and the delta is almost exactly the 32 score-reduction matmuls.
    Non-PE exposure ~1.3us total.
  - "_sg" idealized-stream probe == production: PSUM group cycling, DMA
    deps and ACT interleave cost ~zero. Session spread 275.6-316.3us
    (median ~302) is fleet clock (~2.0-2.2 GHz effective), not kernel.
  - "dr1024_tt" (transposed softmax tail via dma_start_transpose,
    eliminating the 32 sc matmuls): numerically correct (1.780e-2) but
    508us -- 128 XBAR transposes/iter cost ~200us of DMA-queue time to
    save 8us of PE. Kept as a mode for the record; do not ship.

Previous f32r strategy (modes "full_dvesc" etc., kept below and still
selectable): pure data-parallel over batch (B=32 -> 4 per core), zero
collectives. Host-side prep (free, off the HW critical path):
  - c = h @ W_h + b_attn  folded into a per-(batch, hd-tile) bias vector
  - encoder shard pre-transposed to [b, He, S] so the contraction dim He
    lands on SBUF partitions with no on-device transposes
  - W_e pre-tiled to [128, (k, hd, m)] so each [K=128, M=128] lhsT tile is
    a contiguous slice

Device program per core (TileContext), per (batch, 512-col s-block):
  e_projT[hd] = sum_k W_e[k,hd].T @ encT[k]   8x8 f32r matmuls -> PSUM
                (f32r = tf32-like: 1 col/cycle vs 4 for fp32; measured
                 ~0.57 ns/col on this fleet, rel err ~1e-4)
  th[hd]  = tanh(e_projT[hd] + c[b,hd])       ScalarE, PSUM->SBUF, fused bias
  acc    += th[hd] * v[hd]                    VectorE scalar_tensor_tensor
  scores  = ones.T @ acc                      1 matmul: partition-reduce
  exp_row[s-block], partial = Exp(scores)     ScalarE with accum_out
The scores reduction + Exp for block i are deferred into block i+1's
matmul stream so PE never waits on ScalarE/VectorE results.
Per batch: total = sum(partials); the final scaling runs on ScalarE
(fused scale=1/total) in two chunks so the first chunk's output DMA
overlaps the second chunk's scaling.

Softmax skips the max-subtraction: scores are ~N(0, 0.65), |max| < ~4
over 128K samples, exp() is comfortably within f32 range.

Measured: ~490 us per invocation on a quiet chip, ~590 us under fleet
load (per-matmul cost is sequencer-bound ~230 ns quiet / stream-bound
~300 ns busy; 2080 matmuls is the information-theoretic minimum at the
K<=128, N<=512 instruction caps). ~70 TF/s/core effective on the
275 GFLOP GEMM. rel err 9.6e-5.

Rejected-but-measured alternatives (kept as modes for the record):
  - fp8e4 DoubleRow whole-GEMM ("fp8dr"): 1024 instructions, ~1.6x, but
    rel err 1.95e-2 sits on the 2e-2 gate.
  - hybrid He[0:256] fp8-DR + He[256:1024] f32r ("hyb"): 1792
    instructions, ~5-10% faster, rel err 1.00e-2 -- spends half the
    error budget for <10% speed; not worth it.
  - bf16, interleaved accumulation groups, single-group, PSUM buffer
    counts, weight reuse: all within noise of the f32r baseline.

build_nc(n_loop=N) wraps the body in an in-NEFF For_i loop -- used by
test.py to amortize the ~80 ms axon-tunnel dispatch cost when timing.
The graded path is build_nc() defaults.
"""

import os

import numpy as np

import concourse.mybir as mybir
import concourse.tile as tile
from concourse import bacc
from concourse.bass_utils import run_bass_kernel_spmd

N_CORES = 8
L, B, S, He, Hd = 2, 32, 4096, 1024, 1024
BPC = B // N_CORES  # batches per core
KT = He // 128      # contraction tiles
HT = Hd // 128      # hd tiles
SB = 512            # s-block (matmul moving free dim)
NSB = S // SB
F32 = mybir.dt.float32
F32R = mybir.dt.float32r
BF16 = mybir.dt.bfloat16
F16 = mybir.dt.float16

# matmul-input dtype for the big GEMM:
#   "f32r"  - tf32-like, 1 col/cycle
#   "bf16"  - 1 col/cycle
#   "fp8dr" - float8e4 with DoubleRow: K=256 per instruction, 2 multiplies/cycle
MM_DTYPE = "f32r"
# v-dot on DVE ("dvesc" mode) keeps v in f32
VW_F32 = True
F8 = mybir.dt.float8e4
KT2 = KT // 2  # 256-deep contraction tiles for DoubleRow

# --- dr-hybrid mode ("dr<k8>[_ki][_pe]"): first k8 channels fp8e4+DoubleRow,
# rest f32r, s-blocks processed in groups so DR weight tiles are reused
# across consecutive matmuls (amortizes the 2x LDWEIGHTS cost of DR).
#   _ki: k-inner loop order (weights change every matmul) -- A/B probe
#   _pe: emit only the GEMM (no ACT/DVE/softmax) -- PE-bound probe
MODE = os.environ.get("K_MODE", "dr1024")
W_SCALE = 1024.0  # pow2; keeps fp8(W) out of the subnormal zone
DR_GROUPS = ((0, 1, 2), (3, 4, 5), (6, 7))

_NC_CACHE = {}


def _dr_k8(mode):
    return int(mode[2:].split("_")[0])


def _mm_dt():
    return BF16 if MM_DTYPE == "bf16" else F32R


def _emit_body(nc, pools, params, batches=None, mode="full"):
    AFT = mybir.ActivationFunctionType
    enc_pool, th_pool, soft_pool, ep_pool, sc_pool = pools
    encT, out, w_sb, v_sb, c_sb, ones_sb, et_shared, wh, lazy_w = params[:9]
    hyb = "hyb" in mode
    if hyb:
        encT8, w8_sb = params[9:]
    batches = list(range(BPC)) if batches is None else batches
    fp8 = MM_DTYPE == "fp8dr"
    th_dt = F32 if "dvesc" in mode else _mm_dt()

    # flat list of (batch, s-block); scores finalization for block i is
    # deferred into block i+1 so PE never waits on ACT/DVE results
    blocks = [(b, isb) for b in batches for isb in range(NSB)]
    soft = {}    # b -> (exp_row, parts)
    deferred = None  # (b, isb, sc_or_acc, ths)

    def finish_block(dfr):
        b, isb, acc, ths = dfr
        exp_row, parts = soft[b]
        if "dvesc" in mode:
            sc = sc_pool.tile([1, SB], F32, tag="sc")
            nc.tensor.matmul(sc, ones_sb, acc, start=True, stop=True)
        else:
            sc = sc_pool.tile([1, SB], F32, tag="sc")
            for hd in range(HT):
                nc.tensor.matmul(sc, v_sb[:, hd:hd + 1], ths[hd],
                                 start=(hd == 0), stop=(hd == HT - 1))
        nc.scalar.activation(
            exp_row[:, isb * SB:(isb + 1) * SB], sc, AFT.Exp,
            accum_out=parts[:, isb:isb + 1])
        if isb == NSB - 1:
            # batch done: softmax normalization + output
            tot = soft_pool.tile([1, 1], F32, tag="tot")
            nc.vector.tensor_reduce(tot, parts, axis=mybir.AxisListType.X,
                                    op=mybir.AluOpType.add)
            rinv = soft_pool.tile([1, 1], F32, tag="rinv")
            nc.vector.reciprocal(rinv, tot)
            # scale on ScalarE (1.2 GHz vs DVE 0.96 single-lane), in two
            # chunks so the first chunk's output DMA overlaps the second
            # chunk's scaling -- trims the exposed final-batch tail
            half = S // 2
            for c2 in range(2):
                oc = soft_pool.tile([1, half], F32, tag="oc", bufs=4,
                                    name=f"oc_{b}_{c2}")
                nc.scalar.activation(oc, exp_row[:, c2 * half:(c2 + 1) * half],
                                     AFT.Copy, scale=rinv)
                nc.sync.dma_start(out=out[b:b + 1, c2 * half:(c2 + 1) * half],
                                  in_=oc)
            del soft[b]

    for b, isb in blocks:
        if b not in soft:
            soft[b] = (soft_pool.tile([1, S], F32, tag="exp_row",
                                      name=f"exp_row_{b}"),
                       soft_pool.tile([1, NSB], F32, tag="parts",
                                      name=f"parts_{b}"))
        if "compute" in mode:
            et = et_shared
        elif hyb:
            et8 = enc_pool.tile([128, 2, SB], F8, tag="et8")
            nc.sync.dma_start(
                out=et8, in_=encT8[b, :, :, isb * SB:(isb + 1) * SB])
            et = []
            for k in range(KT - 2):
                t = enc_pool.tile([128, SB], F32R, tag="et")
                nc.sync.dma_start(
                    out=t,
                    in_=encT[b, k * 128:(k + 1) * 128, isb * SB:(isb + 1) * SB])
                et.append(t)
        elif fp8:
            et = []
            for k2 in range(KT2):
                t = enc_pool.tile([128, 2, SB], F8, tag="et")
                nc.sync.dma_start(
                    out=t, in_=encT[b, k2, :, :, isb * SB:(isb + 1) * SB])
                et.append(t)
        elif "wet" in mode:
            # wide et: one [128, 2*SB] tile per k covers two s-blocks --
            # halves DMA count and first-use sem waits
            first = (b, isb) == blocks[0]
            if isb % 2 == 0:
                etw = []
                for k in range(KT):
                    t = enc_pool.tile([128, 2 * SB], _mm_dt(), tag="etw",
                                      bufs=12, name=f"etw{k}")
                    nc.sync.dma_start(
                        out=t,
                        in_=encT[b, k * 128:(k + 1) * 128,
                                 isb * SB:(isb + 2) * SB])
                    etw.append(t)
                    if lazy_w and first:
                        ck = HT * 128
                        nc.sync.dma_start(out=w_sb[:, k * ck:(k + 1) * ck],
                                          in_=wh[:, k * ck:(k + 1) * ck])
                _emit_body.etw = etw
            off = (isb % 2) * SB
            et = [t[:, off:off + SB] for t in _emit_body.etw]
        else:
            first = (b, isb) == blocks[0]
            et = []
            for k in range(KT):
                t = enc_pool.tile([128, SB], _mm_dt(), tag="et")
                nc.sync.dma_start(
                    out=t,
                    in_=encT[b, k * 128:(k + 1) * 128, isb * SB:(isb + 1) * SB])
                et.append(t)
                if lazy_w and first:
                    ck = HT * 128
                    nc.sync.dma_start(out=w_sb[:, k * ck:(k + 1) * ck],
                                      in_=wh[:, k * ck:(k + 1) * ck])

        acc = None
        ths = []
        if "g1" in mode:
            first_blk = (b, isb) == blocks[0]
            last_blk = (b, isb) == blocks[-1]
            ep = ep_pool.tile([128, SB], F32, tag="ep1", bufs=1,
                              name="ep_g1")
            for hd in range(HT):
                for k in range(KT):
                    w_tile = w_sb[:, (k * HT + hd) * 128:(k * HT + hd + 1) * 128]
                    nc.tensor.matmul(
                        ep, w_tile, et[k],
                        start=(first_blk and hd == 0 and k == 0),
                        stop=(last_blk and hd == HT - 1 and k == KT - 1))
            if last_blk:
                probe = soft_pool.tile([128, 1], F32, tag="probe")
                nc.scalar.activation(probe, ep[:, 0:1], AFT.Copy)
            continue
        for hd in range(HT):
            ep = ep_pool.tile([128, SB], F32, tag="ep")
            if fp8:
                for k2 in range(KT2):
                    nc.tensor.matmul(
                        ep, w_sb[:, k2 * HT + hd, :, :], et[k2],
                        start=(k2 == 0), stop=(k2 == KT2 - 1),
                        perf_mode=mybir.MatmulPerfMode.DoubleRow)
            elif hyb:
                nc.tensor.matmul(ep, w8_sb[:, hd, :, :], et8,
                                 start=True, stop=False,
                                 perf_mode=mybir.MatmulPerfMode.DoubleRow)
                for k in range(KT - 2):
                    w_tile = w_sb[:, (k * HT + hd) * 128:(k * HT + hd + 1) * 128]
                    nc.tensor.matmul(ep, w_tile, et[k],
                                     start=False, stop=(k == KT - 3))
            else:
                kr = range(KT // 2) if "k4" in mode else range(KT)
                for k in kr:
                    w_tile = w_sb[:, (k * HT + hd) * 128:(k * HT + hd + 1) * 128]
                    nc.tensor.matmul(ep, w_tile, et[k],
                                     start=(k == 0),
                                     stop=(k == list(kr)[-1]))
            if hd == 1 and deferred is not None:
                # PE work for the previous block's scores goes here, long
                # after its inputs are ready
                finish_block(deferred)
                deferred = None
            if "noact" in mode:
                if hd == HT - 1:
                    probe = soft_pool.tile([128, 1], F32, tag="probe")
                    nc.scalar.activation(probe, ep[:, 0:1], AFT.Copy)
                continue
            th = th_pool.tile([128, SB], th_dt, tag="th")
            nc.scalar.activation(
                th, ep, AFT.Tanh, bias=c_sb[:, b * HT + hd: b * HT + hd + 1])
            ths.append(th)
            if "dvesc" in mode:
                if hd == 0:
                    acc = th_pool.tile([128, SB], F32, tag="acc", bufs=3)
                    nc.vector.tensor_scalar_mul(acc, th, v_sb[:, 0:1])
                else:
                    nc.vector.scalar_tensor_tensor(
                        acc, th, v_sb[:, hd:hd + 1], acc,
                        op0=mybir.AluOpType.mult, op1=mybir.AluOpType.add)
        if "noact" in mode:
            continue
        if "dvesc" in mode:
            acc8 = th_pool.tile([128, SB], F32R, tag="acc8", bufs=3)
            nc.scalar.activation(acc8, acc, AFT.Copy)
            acc = acc8
        deferred = (b, isb, acc if "dvesc" in mode else None, ths)
    if deferred is not None and "noact" not in mode:
        finish_block(deferred)


def _emit_dr_body(nc, pools, params, batches=None, mode="dr768"):
    AFT = mybir.ActivationFunctionType
    DR = mybir.MatmulPerfMode.DoubleRow
    enc_pool, th_pool, soft_pool, ep_pool, sc_pool = pools
    (enc8, encf, out, w8_sb, wf_sb, v_sb, ones_sb, c_sb, w8p, wfp,
     lazy_w, ones2_sb) = params
    batches = list(range(BPC)) if batches is None else batches
    k8 = _dr_k8(mode)
    kt2 = k8 // 256
    kf = (He - k8) // 128
    ki = "_ki" in mode
    probe_pe = "_pe" in mode
    tt = "_tt" in mode  # transposed softmax tail: no per-block sc matmuls
    ds = 1.0 / W_SCALE
    groups = ((0, 1, 2, 3, 4, 5), (6, 7)) if "_g6" in mode else DR_GROUPS

    blocks = [(b, g) for b in batches for g in range(len(groups))]
    soft = {}
    finq = []  # deferred (b, isb, accT) score finishers / tt batch ends

    def finish_batch_tt():
        b = finq.pop(0)
        exp_t, parts_t = soft[b]
        # ones2 is [128,128]: every output partition gets the per-isb
        # cross-partition totals in one N=8 matmul
        tot8 = sc_pool.tile([128, NSB], F32, tag="tot8")
        nc.tensor.matmul(tot8, ones2_sb, parts_t, start=True, stop=True)
        tot = soft_pool.tile([128, 1], F32, tag="tot128")
        nc.vector.tensor_reduce(tot, tot8, axis=mybir.AxisListType.X,
                                op=mybir.AluOpType.add)
        rinv = soft_pool.tile([128, 1], F32, tag="rinv128")
        nc.vector.reciprocal(rinv, tot)
        oc = soft_pool.tile([128, S // 128], F32, tag="oct", bufs=4,
                            name=f"oct_{b}")
        nc.scalar.activation(oc, exp_t, mybir.ActivationFunctionType.Copy,
                             scale=rinv)
        nc.sync.dma_start(
            out=out[b:b + 1, :].rearrange("a (c p) -> (a p) c", p=128),
            in_=oc)
        del soft[b]

    def finish_one():
        b, isb, acc = finq.pop(0)
        exp_row, parts = soft[b]
        sc = sc_pool.tile([1, SB], F32, tag="sc")
        nc.tensor.matmul(sc, ones_sb, acc, start=True, stop=True)
        nc.scalar.activation(
            exp_row[:, isb * SB:(isb + 1) * SB], sc, AFT.Exp,
            accum_out=parts[:, isb:isb + 1])
        if isb == NSB - 1:
            tot = soft_pool.tile([1, 1], F32, tag="tot")
            nc.vector.tensor_reduce(tot, parts, axis=mybir.AxisListType.X,
                                    op=mybir.AluOpType.add)
            rinv = soft_pool.tile([1, 1], F32, tag="rinv")
            nc.vector.reciprocal(rinv, tot)
            half = S // 2
            for c2 in range(2):
                oc = soft_pool.tile([1, half], F32, tag="oc", bufs=4,
                                    name=f"oc_{b}_{c2}")
                nc.scalar.activation(oc, exp_row[:, c2 * half:(c2 + 1) * half],
                                     AFT.Copy, scale=rinv)
                nc.sync.dma_start(out=out[b:b + 1, c2 * half:(c2 + 1) * half],
                                  in_=oc)
            del soft[b]

    for b, g in blocks:
        isbs = groups[g]
        ni = len(isbs)
        s0 = isbs[0] * SB
        gw = ni * SB
        if b not in soft and not probe_pe:
            if tt:
                soft[b] = (soft_pool.tile([128, S // 128], F32, tag="exp_t",
                                          name=f"exp_t_{b}"),
                           soft_pool.tile([128, NSB], F32R, tag="parts_t",
                                          name=f"parts_t_{b}"))
            else:
                soft[b] = (soft_pool.tile([1, S], F32, tag="exp_row",
                                          name=f"exp_row_{b}"),
                           soft_pool.tile([1, NSB], F32, tag="parts",
                                          name=f"parts_{b}"))
        first = (b, g) == blocks[0]
        et8 = []
        for k2 in range(kt2):
            t = enc_pool.tile([128, 2, gw], F8, tag=f"et8_{k2}", bufs=3)
            nc.sync.dma_start(out=t, in_=enc8[b, k2, :, :, s0:s0 + gw])
            et8.append(t)
            if lazy_w and first:
                nc.sync.dma_start(
                    out=w8_sb[:, k2 * HT:(k2 + 1) * HT, :, :],
                    in_=w8p[:, k2 * HT:(k2 + 1) * HT, :, :])
        etf = []
        for k in range(kf):
            t = enc_pool.tile([128, gw], F32R, tag=f"etf_{k}", bufs=3)
            nc.sync.dma_start(out=t, in_=encf[b, k * 128:(k + 1) * 128,
                                             s0:s0 + gw])
            etf.append(t)
            if lazy_w and first:
                ck = HT * 128
                nc.sync.dma_start(out=wf_sb[:, k * ck:(k + 1) * ck],
                                  in_=wfp[:, k * ck:(k + 1) * ck])
        if "_sg" in mode:
            # pure-stream probe: one PSUM accumulation group, constant
            # weight+rhs, no ACT/DVE -- measures the intrinsic DR matmul
            # stream floor (PSUM cycling + dep-wait cost excluded)
            first_blk = (b, g) == blocks[0]
            last_blk = (b, g) == blocks[-1]
            epg = ep_pool.tile([128, SB], F32, tag="epg", bufs=1,
                               name="ep_sg")
            nmm_all = HT * ni * kt2
            mi = 0
            for hd in range(HT):
                for k2 in range(kt2):
                    for i in range(ni):
                        nc.tensor.matmul(
                            epg, w8_sb[:, 0, :, :], et8[0][:, :, 0:SB],
                            start=(first_blk and mi == 0),
                            stop=(last_blk and mi == nmm_all - 1),
                            perf_mode=DR)
                        mi += 1
            if last_blk:
                pr = soft_pool.tile([128, 1], F32, tag="probe")
                nc.scalar.activation(pr, epg[:, 0:1], AFT.Copy)
            continue
        accs = [None] * ni
        for hd in range(HT):
            eps = []
            for i in range(ni):
                ep = ep_pool.tile([128, SB], F32, tag="ep", name=f"ep{i}")
                eps.append(ep)
            nmm = kt2 + kf

            def mm(i, mi, k2=None, k=None):
                if k2 is not None:
                    if "_1w" in mode:  # probe: constant weight+rhs tiles
                        nc.tensor.matmul(
                            eps[i], w8_sb[:, 0, :, :], et8[0][:, :, 0:SB],
                            start=(mi == 0), stop=(mi == nmm - 1),
                            perf_mode=DR)
                        return
                    nc.tensor.matmul(
                        eps[i], w8_sb[:, k2 * HT + hd, :, :],
                        et8[k2][:, :, i * SB:(i + 1) * SB],
                        start=(mi == 0), stop=(mi == nmm - 1), perf_mode=DR)
                else:
                    nc.tensor.matmul(
                        eps[i],
                        wf_sb[:, (k * HT + hd) * 128:(k * HT + hd + 1) * 128],
                        etf[k][:, i * SB:(i + 1) * SB],
                        start=(mi == 0), stop=(mi == nmm - 1))

            if ki:  # weights swapped every matmul (A/B probe)
                for i in range(ni):
                    for mi, k2 in enumerate(range(kt2)):
                        mm(i, mi, k2=k2)
                    for mi, k in enumerate(range(kf)):
                        mm(i, kt2 + mi, k=k)
            else:   # weight tile reused across the group's s-blocks
                for mi, k2 in enumerate(range(kt2)):
                    for i in range(ni):
                        mm(i, mi, k2=k2)
                for mi, k in enumerate(range(kf)):
                    for i in range(ni):
                        mm(i, kt2 + mi, k=k)

            if probe_pe:
                if (b, g) == blocks[-1] and hd == HT - 1:
                    pr = soft_pool.tile([128, 1], F32, tag="probe")
                    nc.scalar.activation(pr, eps[-1][:, 0:1], AFT.Copy)
                continue
            if hd >= 1 and finq:
                finish_batch_tt() if tt else finish_one()
            for i in range(ni):
                th = th_pool.tile([128, SB], F32, tag="th")
                nc.scalar.activation(
                    th, eps[i], AFT.Tanh,
                    bias=c_sb[:, b * HT + hd:b * HT + hd + 1], scale=ds)
                if hd == 0:
                    acc = th_pool.tile([128, SB], F32 if tt else F32R,
                                       tag="acc", bufs=8, name=f"acc{i}")
                    accs[i] = acc
                    nc.vector.tensor_scalar_mul(accs[i], th, v_sb[:, 0:1])
                else:
                    nc.vector.scalar_tensor_tensor(
                        accs[i], th, v_sb[:, hd:hd + 1], accs[i],
                        op0=mybir.AluOpType.mult, op1=mybir.AluOpType.add)
        if probe_pe:
            continue
        if tt:
            exp_t, parts_t = soft[b]
            for i, isb in enumerate(isbs):
                a16 = th_pool.tile([128, SB], F16, tag="a16", bufs=4,
                                   name=f"a16_{i}")
                nc.scalar.activation(a16, accs[i],
                                     mybir.ActivationFunctionType.Copy)
                ttile = th_pool.tile([128, SB // 128, 128], F16, tag="ttile",
                                     bufs=4, name=f"ttile_{i}")
                for c in range(SB // 128):
                    nc.sync.dma_start_transpose(
                        ttile[:, c, :], a16[:, c * 128:(c + 1) * 128])
                st = th_pool.tile([128, SB // 128], F32, tag="st", bufs=4,
                                  name=f"st_{i}")
                with nc.allow_low_precision(
                        reason="fp16 transpose staging adds ~0.05% to "
                               "scores; fp32 reduce output"):
                    nc.vector.tensor_reduce(st, ttile,
                                            axis=mybir.AxisListType.X,
                                            op=mybir.AluOpType.add)
                with nc.allow_low_precision(
                        reason="per-block exp partials rounded to f32r for "
                               "the broadcast matmul; 2^-12 relative"):
                    nc.scalar.activation(
                        exp_t[:, isb * 4:(isb + 1) * 4], st,
                        mybir.ActivationFunctionType.Exp,
                        accum_out=parts_t[:, isb:isb + 1])
            if isbs[-1] == NSB - 1:
                finq.append(b)
        else:
            for i, isb in enumerate(isbs):
                finq.append((b, isb, accs[i]))
    while finq:
        finish_batch_tt() if tt else finish_one()


def _build_nc_dr(n_loop, batches, mode, ep_bufs):
    k8 = _dr_k8(mode)
    kt2 = k8 // 256
    kf = (He - k8) // 128
    nc = bacc.Bacc(trn_type="TRN2", target_bir_lowering=False, debug=False,
                   num_devices=N_CORES)
    enc8 = nc.declare_dram_parameter("enc8", [BPC, kt2, 128, 2, S], F8,
                                     isOutput=False)
    w8p = nc.declare_dram_parameter("w8", [128, kt2 * HT, 2, 128], F8,
                                    isOutput=False)
    encf = wfp = None
    if kf:
        encf = nc.declare_dram_parameter("encf", [BPC, kf * 128, S], F32R,
                                         isOutput=False)
        wfp = nc.declare_dram_parameter("wf", [128, kf * HT * 128], F32R,
                                        isOutput=False)
    cb = nc.declare_dram_parameter("cb", [128, BPC * HT], F32, isOutput=False)
    vw = nc.declare_dram_parameter("vw", [128, HT], F32, isOutput=False)
    onesp = nc.declare_dram_parameter("ones", [128, 1], F32R, isOutput=False)
    ones2p = nc.declare_dram_parameter("ones2", [128, 128], F32R,
                                       isOutput=False)
    out = nc.declare_dram_parameter("out", [BPC, S], F32, isOutput=True)

    with tile.TileContext(nc) as tc:
        with (
            tc.tile_pool(name="consts", bufs=1) as consts,
            tc.tile_pool(name="enc", bufs=2) as enc_pool,
            tc.tile_pool(name="th", bufs=10) as th_pool,
            tc.tile_pool(name="soft", bufs=2) as soft_pool,
            tc.tile_pool(name="ep", bufs=ep_bufs, space="PSUM") as ep_pool,
            tc.tile_pool(name="sc", bufs=2, space="PSUM") as sc_pool,
        ):
            lazy_w = n_loop == 1
            w8_sb = consts.tile([128, kt2 * HT, 2, 128], F8)
            if not lazy_w:
                nc.sync.dma_start(out=w8_sb, in_=w8p[:])
            wf_sb = None
            if kf:
                wf_sb = consts.tile([128, kf * HT * 128], F32R)
                if not lazy_w:
                    ck = HT * 128
                    for k in range(kf):
                        nc.sync.dma_start(out=wf_sb[:, k * ck:(k + 1) * ck],
                                          in_=wfp[:, k * ck:(k + 1) * ck])
            v_sb = consts.tile([128, HT], F32)
            nc.sync.dma_start(out=v_sb, in_=vw[:])
            ones_sb = consts.tile([128, 1], F32R)
            nc.sync.dma_start(out=ones_sb, in_=onesp[:])
            ones2_sb = consts.tile([128, 128], F32R)
            nc.sync.dma_start(out=ones2_sb, in_=ones2p[:])
            c_sb = consts.tile([128, BPC * HT], F32)
            nc.sync.dma_start(out=c_sb, in_=cb[:])

            pools = (enc_pool, th_pool, soft_pool, ep_pool, sc_pool)
            params = (enc8, encf, out, w8_sb, wf_sb, v_sb, ones_sb, c_sb,
                      w8p, wfp, lazy_w, ones2_sb)
            if n_loop == 1:
                _emit_dr_body(nc, pools, params, batches, mode)
            else:
                with tc.For_i(0, n_loop, 1):
                    _emit_dr_body(nc, pools, params, batches, mode)
    nc.compile()
    return nc


def prepare_in_maps_dr(hidden, encoder_outputs, W_attn, b_attn, v_w,
                       mode=None):
    import ml_dtypes
    f8 = ml_dtypes.float8_e4m3
    mode = MODE if mode is None else mode
    k8 = _dr_k8(mode)
    kt2 = k8 // 256
    kf = (He - k8) // 128
    hidden = np.ascontiguousarray(np.asarray(hidden, dtype=np.float32))
    enc = np.asarray(encoder_outputs, dtype=np.float32)
    W_attn = np.asarray(W_attn, dtype=np.float32)
    b_attn = np.asarray(b_attn, dtype=np.float32)
    v_w = np.asarray(v_w, dtype=np.float32)

    h = hidden[-1]
    W_h = W_attn[:He]
    W_e = W_attn[He:]
    c = (h @ W_h + b_attn).astype(np.float32)   # [B, Hd]

    # w8[p, k2*HT+hd, j, m] = SW * W_e[k2*256 + j*128 + p, hd*128 + m]
    w8 = np.ascontiguousarray(
        (W_e[:k8] * W_SCALE).reshape(kt2, 2, 128, HT, 128)
        .transpose(2, 0, 3, 1, 4).reshape(128, kt2 * HT, 2, 128).astype(f8))
    wf = None
    if kf:
        # wf[p, (k*HT+hd)*128+m] = SW * W_e[k8 + k*128 + p, hd*128 + m]
        wf = np.ascontiguousarray(
            (W_e[k8:] * W_SCALE).reshape(kf, 128, HT, 128)
            .transpose(1, 0, 2, 3).reshape(128, -1).astype(np.float32))
    vw = np.ascontiguousarray(v_w.reshape(HT, 128).T.astype(np.float32))

    in_maps = []
    for ci in range(N_CORES):
        bsl = slice(ci * BPC, (ci + 1) * BPC)
        # enc8[b, k2, p, j, s] = enc[b, s, k2*256 + j*128 + p]
        enc8 = np.ascontiguousarray(
            enc[bsl, :, :k8].reshape(BPC, S, kt2, 2, 128)
            .transpose(0, 2, 4, 3, 1).astype(f8))
        cbm = np.ascontiguousarray(
            c[bsl].reshape(BPC, HT, 128).transpose(2, 0, 1).reshape(128, -1))
        m = {"enc8": enc8, "w8": w8, "cb": cbm, "vw": vw,
             "ones": np.ones((128, 1), np.float32),
             "ones2": np.ones((128, 128), np.float32)}
        if kf:
            m["encf"] = np.ascontiguousarray(
                enc[bsl, :, k8:].transpose(0, 2, 1).astype(np.float32))
            m["wf"] = wf
        in_maps.append(m)
    return in_maps


def build_nc(n_loop=1, batches=None, mode=None, ep_bufs=None):
    mode = MODE if mode is None else mode
    if ep_bufs is None:
        ep_bufs = 6 if mode.startswith("dr") else 4
    key = (MM_DTYPE, n_loop, tuple(batches) if batches else None, mode, ep_bufs)
    if key in _NC_CACHE:
        return _NC_CACHE[key]
    if mode.startswith("dr"):
        nc = _build_nc_dr(n_loop, batches, mode, ep_bufs)
        _NC_CACHE[key] = nc
        return nc
    return _build_nc_orig(n_loop, batches, mode, ep_bufs)


def _build_nc_orig(n_loop=1, batches=None, mode="full_dvesc", ep_bufs=4):
    key = (MM_DTYPE, n_loop, tuple(batches) if batches else None, mode, ep_bufs)
    if key in _NC_CACHE:
        return _NC_CACHE[key]
    nc = bacc.Bacc(trn_type="TRN2", target_bir_lowering=False, debug=False,
                   num_devices=N_CORES)
    if "hyb" in mode:
        encT = nc.declare_dram_parameter("encT", [BPC, He - 256, S], F32R,
                                         isOutput=False)
        wh = nc.declare_dram_parameter("wh", [128, (KT - 2) * HT * 128], F32R,
                                       isOutput=False)
        encT8 = nc.declare_dram_parameter("encT8", [BPC, 128, 2, S], F8,
                                          isOutput=False)
        wh8 = nc.declare_dram_parameter("wh8", [128, HT, 2, 128], F8,
                                        isOutput=False)
    elif MM_DTYPE == "fp8dr":
        encT = nc.declare_dram_parameter("encT", [BPC, KT2, 128, 2, S], F8,
                                         isOutput=False)
        wh = nc.declare_dram_parameter("wh", [128, KT2 * HT, 2, 128], F8,
                                       isOutput=False)
        encT8 = wh8 = None
    else:
        encT = nc.declare_dram_parameter("encT", [BPC, He, S], _mm_dt(),
                                         isOutput=False)
        wh = nc.declare_dram_parameter("wh", [128, KT * HT * 128], _mm_dt(),
                                       isOutput=False)
    cb = nc.declare_dram_parameter("cb", [128, BPC * HT], F32, isOutput=False)
    vdt = F32 if "dvesc" in mode else _mm_dt()
    vw = nc.declare_dram_parameter("vw", [128, HT], vdt, isOutput=False)
    onesp = nc.declare_dram_parameter("ones", [128, 1], F32R, isOutput=False)
    out = nc.declare_dram_parameter("out", [BPC, S], F32, isOutput=True)

    with tile.TileContext(nc) as tc:
        with (
            tc.tile_pool(name="consts", bufs=1) as consts,
            tc.tile_pool(name="enc", bufs=24) as enc_pool,
            tc.tile_pool(name="th", bufs=10) as th_pool,
            tc.tile_pool(name="soft", bufs=2) as soft_pool,
            tc.tile_pool(name="ep", bufs=ep_bufs, space="PSUM") as ep_pool,
            tc.tile_pool(name="sc", bufs=2, space="PSUM") as sc_pool,
        ):
            lazy_w = n_loop == 1 and MM_DTYPE != "fp8dr" and "hyb" not in mode
            w8_sb = None
            if "hyb" in mode:
                w8_sb = consts.tile([128, HT, 2, 128], F8)
                nc.sync.dma_start(out=w8_sb, in_=wh8[:])
            if "hyb" in mode:
                w_sb = consts.tile([128, (KT - 2) * HT * 128], F32R)
                nc.sync.dma_start(out=w_sb, in_=wh[:])
            elif MM_DTYPE == "fp8dr":
                w_sb = consts.tile([128, KT2 * HT, 2, 128], F8)
                for k2 in range(KT2):
                    nc.sync.dma_start(out=w_sb[:, k2 * HT:(k2 + 1) * HT, :, :],
                                      in_=wh[:, k2 * HT:(k2 + 1) * HT, :, :])
            else:
                w_sb = consts.tile([128, KT * HT * 128], _mm_dt())
                if not lazy_w:
                    ck = HT * 128
                    for k in range(KT):
                        nc.sync.dma_start(out=w_sb[:, k * ck:(k + 1) * ck],
                                          in_=wh[:, k * ck:(k + 1) * ck])
            v_sb = consts.tile([128, HT], vdt)
            nc.sync.dma_start(out=v_sb, in_=vw[:])
            ones_sb = consts.tile([128, 1], F32R)
            nc.sync.dma_start(out=ones_sb, in_=onesp[:])
            c_sb = consts.tile([128, BPC * HT], F32)
            nc.sync.dma_start(out=c_sb, in_=cb[:])

            pools = (enc_pool, th_pool, soft_pool, ep_pool, sc_pool)
            et_shared = None
            if "compute" in mode:
                et_shared = []
                for k in range(KT):
                    t = consts.tile([128, SB], _mm_dt(), tag=f"etc{k}")
                    nc.sync.dma_start(out=t, in_=encT[0, k * 128:(k + 1) * 128, 0:SB])
                    et_shared.append(t)
            params = (encT, out, w_sb, v_sb, c_sb, ones_sb, et_shared,
                      wh, lazy_w)
            if "hyb" in mode:
                params = params + (encT8, w8_sb)
            if n_loop == 1:
                _emit_body(nc, pools, params, batches, mode)
            else:
                with tc.For_i(0, n_loop, 1):
                    _emit_body(nc, pools, params, batches, mode)
    nc.compile()
    _NC_CACHE[key] = nc
    return nc


def _np_mm_dt():
    if MM_DTYPE == "bf16":
        import ml_dtypes
        return ml_dtypes.bfloat16
    if MM_DTYPE == "fp8dr":
        import ml_dtypes
        return ml_dtypes.float8_e4m3
    return np.float32


def prepare_in_maps(hidden, encoder_outputs, W_attn, b_attn, v_w,
                    hyb=False):
    if MODE.startswith("dr"):
        return prepare_in_maps_dr(hidden, encoder_outputs, W_attn, b_attn,
                                  v_w)
    mmdt = _np_mm_dt()
    hidden = np.ascontiguousarray(np.asarray(hidden, dtype=np.float32))
    enc = np.asarray(encoder_outputs, dtype=np.float32)
    W_attn = np.asarray(W_attn, dtype=np.float32)
    b_attn = np.asarray(b_attn, dtype=np.float32)
    v_w = np.asarray(v_w, dtype=np.float32)

    h = hidden[-1]                      # [B, He]
    W_h = W_attn[:He]                   # [He, Hd]
    W_e = W_attn[He:]                   # [He, Hd]
    c = (h @ W_h + b_attn).astype(np.float32)   # [B, Hd]

    wh8 = None
    if hyb:
        import ml_dtypes
        f8 = ml_dtypes.float8_e4m3
        # fp8 part: He[0:256]; wh8[p, hd, s, m] = W_e[s*128+p, hd*128+m]
        wh8 = np.ascontiguousarray(
            W_e[:256].reshape(2, 128, HT, 128).transpose(1, 2, 0, 3)
            .reshape(128, HT, 2, 128).astype(f8))
        wh = np.ascontiguousarray(
            W_e[256:].reshape(KT - 2, 128, HT, 128).transpose(1, 0, 2, 3)
            .reshape(128, -1).astype(np.float32))
    elif MM_DTYPE == "fp8dr":
        # wh[p, k2*HT+hd, s, m] = W_e[k2*256 + s*128 + p, hd*128+m]
        wh = np.ascontiguousarray(
            W_e.reshape(KT2, 2, 128, HT, 128).transpose(2, 0, 3, 1, 4)
            .reshape(128, KT2 * HT, 2, 128).astype(mmdt))
    else:
        # wh[p, (k*HT+hd)*128+m] = W_e[k*128+p, hd*128+m]
        wh = np.ascontiguousarray(
            W_e.reshape(KT, 128, HT, 128).transpose(1, 0, 2, 3).reshape(128, -1)
            .astype(mmdt))
    # vw[p, hd] = v_w[hd*128+p]
    vw_dt = np.float32 if (VW_F32 or MM_DTYPE == "fp8dr") else mmdt
    vw = np.ascontiguousarray(v_w.reshape(HT, 128).T.astype(vw_dt))

    in_maps = []
    for ci in range(N_CORES):
        bsl = slice(ci * BPC, (ci + 1) * BPC)
        encT8 = None
        if hyb:
            import ml_dtypes
            f8 = ml_dtypes.float8_e4m3
            # encT8[b, p, s, n] = enc[b, n, s*128 + p] for He[0:256]
            encT8 = np.ascontiguousarray(
                enc[bsl, :, :256].reshape(BPC, S, 2, 128)
                .transpose(0, 3, 2, 1).astype(f8))
            encT = np.ascontiguousarray(
                enc[bsl, :, 256:].transpose(0, 2, 1).astype(np.float32))
        elif MM_DTYPE == "fp8dr":
            # encT[b, k2, p, s, n] = enc[b, n, k2*256 + s*128 + p]
            encT = np.ascontiguousarray(
                enc[bsl].reshape(BPC, S, KT2, 2, 128)
                .transpose(0, 2, 4, 3, 1).astype(mmdt))
        else:
            encT = np.ascontiguousarray(
                enc[bsl].transpose(0, 2, 1).astype(mmdt))  # [BPC, He, S]
        cb = np.ascontiguousarray(
            c[bsl].reshape(BPC, HT, 128).transpose(2, 0, 1).reshape(128, -1))
        m = {"encT": encT, "wh": wh, "cb": cb, "vw": vw,
             "ones": np.ones((128, 1), np.float32)}
        if hyb:
            m["encT8"] = encT8
            m["wh8"] = wh8
        in_maps.append(m)
    return in_maps


def kernel(hidden, encoder_outputs, W_attn, b_attn, v_w):
    nc = build_nc()
    in_maps = prepare_in_maps(hidden, encoder_outputs, W_attn, b_attn, v_w)
    res = run_bass_kernel_spmd(nc, in_maps, core_ids=list(range(N_CORES)))
    return np.concatenate([res.results[i]["out"] for i in range(N_CORES)],
                          axis=0)

